# revision 9
# baseline (speedup 1.0000x reference)
"""CustomGAT (gnn_message_passing) Trainium2 kernel — 8-core SPMD, v2.

Design (dst-resident, free-axis edge layout):
  * Host: order nodes by (lo-deg, hi-deg) lexsort so consecutive 128-node
    dst blocks have near-equal per-bin in-degrees (few % slot padding).
    Edges of dst p live on partition p along the free axis (j-slots);
    per-edge src table rows split into two int16-addressable bins
    (rows < 32768, base 0; rows >= 32768, base 17408).
  * Device phase A (replicated on all cores): projection table rows
    [xp bf16 x256 | al bf16 x8 | ar bf16 x8 | pad to 768B] via fp32r
    matmuls; one ACT copy per tile PSUM->SBUF. Pad rows 0 and 50001 get
    al=ar=-80 so padded gather slots contribute exp(lrelu(-160+..)) ~ 0.
  * Device phase B (49 dst blocks per core): one m=1 "self" gather call
    brings each dst's own row (= the self-loop edge AND the block's ar
    column); real edges batch-gathered by the GPSIMD dma_gather ucode
    round-robined over 4 SWDGE queues (the descriptor-generation
    bottleneck measured 8.5 ns/row on one queue, ~3.3 ns/row on four).
    Logits al+ar on DVE; exp(leaky_relu(x)) = max(exp(x), exp(0.2x)) via
    two ACT exps + DVE max; messages xp*t on DVE; j-slot reduction via
    identity matmuls accumulating into PSUM [128, 264] whose last 8 cols
    collect the softmax denominator; one normalize per block.
  * Host: inverse-permute per-core shards to the full [N, 256] output.
"""

import numpy as np

# ---------------------------------------------------------------- constants
H = 8
C = 32
HC = H * C  # 256
IN = 256
P = 128
ROW = 384   # bf16 row: [xp 0:256 | al 256:264 | ar 264:272 | pad]
RCOL = 272  # used row columns
PSROW = 272
SLOTS = 50176  # 392 * 128; row 0 = lo pad, rows 50001+ = hi pad area
NBLK = SLOTS // P  # 392
NLO = 256   # blocks 0..255 have rows < 32768
LOWMAX = 32768
HIBASE = 17408  # hi bin rows [32768, 50176) addressed as row - 17408
PADLO = 0
PADHI = 50001
MAXJ = 8  # j-slots per dma_gather call (1024-index ucode cap)
NQ = 4    # SWDGE queues
G8 = 8    # phase A tiles per DMA group


# ---------------------------------------------------------------- tile patch
def _install_tile_patch():
    """The axon-path walrus rejects >2 sync waits on one instruction; split
    the TileContext tail-drain waits into one carrier drain per proc."""
    import concourse.tile as tile
    from concourse.vector_clock import ScopedClock, VectorClock

    if getattr(tile.TileContext, "_drain_patch_installed", False):
        return

    def _drain_and_barrier(self, tick_clock, wait_clock):
        gc = tick_clock.global_clock
        n = len(gc)
        for p in range(n):
            if gc[p] == 0:
                continue
            req = VectorClock([gc[q] if q == p else 0 for q in range(n)])
            d = self.nc.sync.drain()
            wait_clock.add_sem_waits(d.ins, ScopedClock({None: req}))
        self.nc.all_engine_barrier()
        assert self.sems is not None
        popped = self.nc._tile_sem_poison_stack.pop()
        assert popped is self._sem_poison
        self.nc.clear_and_free_semaphores(list(self.sems.allocated().values()))
        self.nc.all_engine_barrier()

    tile.TileContext._drain_and_barrier = _drain_and_barrier
    tile.TileContext._drain_patch_installed = True


# ---------------------------------------------------------------- host prep
def _idx16(vals, m):
    """Encode row indices for dma_gather: [128, m*8] int16, index k at
    [k%16 (+16*rep), k//16], replicated for the 8 Q7 cores."""
    enc = vals.astype(np.int64).astype(np.int16)
    a = enc.reshape(m * 8, 16).T  # [16, m*8]
    return np.tile(a, (8, 1))


def _calls_of(k):
    return [min(MAXJ, k - s) for s in range(0, k, MAXJ)]


def _preprocess(x, edge_index, W, attn_l, attn_r, n_cores):
    import ml_dtypes

    N = x.shape[0]
    src = np.asarray(edge_index[0]).astype(np.int64)
    dst = np.asarray(edge_index[1]).astype(np.int64)

    deg = np.bincount(dst, minlength=N)
    # iterate the (lo-deg, hi-deg) lexsort toward a fixed point
    order = np.argsort(-deg, kind="stable")
    row_of = np.empty(N, np.int64)
    row_of[order] = 1 + np.arange(N)  # row 0 reserved for lo-pad
    for _ in range(3):
        lo = np.bincount(dst[row_of[src] < LOWMAX], minlength=N)
        hi = deg - lo
        order = np.lexsort((hi, lo))
        row_of[order] = 1 + np.arange(N)

    srow = row_of[src]
    drow = row_of[dst]
    ishi = (srow >= LOWMAX).astype(np.int64)

    eblk = drow // P
    epart = drow % P
    cnt = np.zeros((NBLK, P, 2), np.int64)
    np.add.at(cnt, (eblk, epart, ishi), 1)
    klo_b = cnt[:, :, 0].max(axis=1)
    khi_b = cnt[:, :, 1].max(axis=1)
    w_b = klo_b + khi_b

    # positions: 0..31 lo-group (blocks 0..255), 32..48 hi-group.
    # 8 CONSECUTIVE lexsorted blocks share a position so their (klo, khi)
    # nearly coincide and the per-position max costs ~1 extra slot.
    bpc = NBLK // n_cores  # 49
    npos_lo = NLO // n_cores  # 32
    blk_at = np.arange(NBLK, dtype=np.int64).reshape(bpc, n_cores)

    # uniform per-position call shapes (max over the 8 cores' blocks);
    # the self row is slot j=0 of the block's own bin (lo for pos<npos_lo)
    klo_pos = klo_b[blk_at].max(axis=1)
    khi_pos = khi_b[blk_at].max(axis=1)
    calls_pos = []  # per pos: ordered list of (binflag, m); self bin first
    for i in range(bpc):
        if i < npos_lo:
            klo_pos[i] += 1
        else:
            khi_pos[i] += 1
        cl = [(0, m) for m in _calls_of(int(klo_pos[i]))]
        ch = [(1, m) for m in _calls_of(int(khi_pos[i]))]
        calls_pos.append(cl + ch if i < npos_lo else ch + cl)
    tot_slots = int(sum(klo_pos[i] + khi_pos[i] for i in range(bpc)) * P)

    # per-(block, bin) gather index grids [k, P], padded by bin pad row
    idx_lo = {b: np.full((int(klo_pos[pos]), P), PADLO, np.int64)
              for pos in range(bpc) for b in blk_at[pos]}
    idx_hi = {b: np.full((int(khi_pos[pos]), P), PADHI - HIBASE, np.int64)
              for pos in range(bpc) for b in blk_at[pos]}
    # j index per edge: cumcount within (block, part, bin)
    key = (eblk * P + epart) * 2 + ishi
    eorder = np.argsort(key, kind="stable")
    ks = key[eorder]
    grp_start = np.flatnonzero(np.concatenate([[True], ks[1:] != ks[:-1]]))
    sizes = np.diff(np.concatenate([grp_start, [len(ks)]]))
    jvals = np.arange(len(ks)) - np.repeat(grp_start, sizes)
    ej = np.empty(len(ks), np.int64)
    ej[eorder] = jvals

    # edges land at j (+1 when their bin carries the self slot at j=0)
    lo_m = ishi == 0
    bs, ps, js, rs = eblk[lo_m], epart[lo_m], ej[lo_m], srow[lo_m]
    shift = (bs < NLO).astype(np.int64)
    for b in range(NBLK):
        m = bs == b
        idx_lo[b][js[m] + shift[m], ps[m]] = rs[m]
    hi_m = ishi == 1
    bs, ps, js, rs = eblk[hi_m], epart[hi_m], ej[hi_m], srow[hi_m] - HIBASE
    shift = (bs >= NLO).astype(np.int64)
    for b in range(NBLK):
        m = bs == b
        idx_hi[b][js[m] + shift[m], ps[m]] = rs[m]
    for b in range(NBLK):
        base = b * P
        selfidx = np.arange(base, base + P, dtype=np.int64)
        if b < NLO:
            idx_lo[b][0] = selfidx
        else:
            idx_hi[b][0] = selfidx - HIBASE

    # per-core concatenated idx16, self-bin calls first per position
    core_idx = []
    for c in range(n_cores):
        chunks = []
        for pos in range(bpc):
            b = int(blk_at[pos, c])
            parts = [(idx_lo[b], int(klo_pos[pos])),
                     (idx_hi[b], int(khi_pos[pos]))]
            if b >= NLO:
                parts = parts[::-1]
            for grid, k in parts:
                o = 0
                for m in _calls_of(k):
                    chunks.append(_idx16(grid[o:o + m].reshape(-1), m))
                    o += m
        core_idx.append(np.ascontiguousarray(np.concatenate(chunks, axis=1)))
    tot8 = core_idx[0].shape[1]

    # weights: Wcat [256, 272] = [W.T | B_l | B_r]
    W = np.asarray(W, dtype=np.float32)
    attn_l = np.asarray(attn_l, dtype=np.float32).reshape(H, C)
    attn_r = np.asarray(attn_r, dtype=np.float32).reshape(H, C)
    A_l = np.zeros((HC, H), dtype=np.float32)
    A_r = np.zeros((HC, H), dtype=np.float32)
    for h in range(H):
        A_l[h * C: (h + 1) * C, h] = attn_l[h]
        A_r[h * C: (h + 1) * C, h] = attn_r[h]
    WT = np.ascontiguousarray(W.T)
    wcat = np.concatenate([WT, WT @ A_l, WT @ A_r], axis=1)  # [256, 272]
    wcat = np.ascontiguousarray(wcat.reshape(2, P, PSROW)).astype(np.float32)

    # x tiles: [T, 2, 128(in), 128(node)] so xp rows emerge in table order
    T = SLOTS // P
    x_slot = np.zeros((SLOTS, IN), dtype=np.float32)
    x_slot[row_of] = np.asarray(x, dtype=np.float32)
    xt = np.ascontiguousarray(
        x_slot.reshape(T, P, 2, P).transpose(0, 2, 3, 1), dtype=np.float32)

    alpad = np.full((2, 16), -80.0, dtype=ml_dtypes.bfloat16)
    ident = np.eye(P, dtype=ml_dtypes.bfloat16)

    meta = dict(n_cores=n_cores, T=T, bpc=bpc, row_of=row_of, blk_at=blk_at,
                calls_pos=calls_pos, tot8=tot8, tot_slots=tot_slots)
    shared = dict(xt=xt, wcat=wcat, alpad=alpad, ident=ident)
    per_core = [dict(idx=core_idx[c]) for c in range(n_cores)]
    return meta, shared, per_core


# ---------------------------------------------------------------- device IR
def _build_program(meta):
    import concourse.bacc as bacc
    import concourse.tile as tile
    from concourse import mybir

    _install_tile_patch()

    T, bpc, tot8 = meta["T"], meta["bpc"], meta["tot8"]
    calls_pos = meta["calls_pos"]
    n_cores = meta["n_cores"]
    npos_lo = NLO // n_cores
    f32 = mybir.dt.float32
    bf16 = mybir.dt.bfloat16
    i16 = mybir.dt.int16
    f32r = mybir.dt.float32r
    Alu = mybir.AluOpType
    Act = mybir.ActivationFunctionType

    nc = bacc.Bacc("TRN2", target_bir_lowering=False, debug=False,
                   num_devices=n_cores, num_swdge_queues=NQ)
    xt_in = nc.dram_tensor("xt", [T, 2, P, P], f32r, kind="ExternalInput").ap()
    wcat_in = nc.dram_tensor("wcat", [2, P, PSROW], f32r,
                             kind="ExternalInput").ap()
    alpad_in = nc.dram_tensor("alpad", [2, 16], bf16, kind="ExternalInput").ap()
    ident_in = nc.dram_tensor("ident", [P, P], bf16, kind="ExternalInput").ap()
    idx_in = nc.dram_tensor("idx", [P, tot8], i16, kind="ExternalInput").ap()
    out_ex = nc.dram_tensor("out", [bpc * P, HC], bf16,
                            kind="ExternalOutput").ap()

    with tile.TileContext(nc) as tc:
        with (
            tc.tile_pool(name="const", bufs=1) as cpool,
            tc.tile_pool(name="dram", bufs=1, space="DRAM") as dpool,
        ):
            table = dpool.tile([SLOTS, ROW], bf16)
            wc0 = cpool.tile([P, PSROW], f32r, tag="wc0")
            wc1 = cpool.tile([P, PSROW], f32r, tag="wc1")
            nc.sync.dma_start(wc0[:], wcat_in[0])
            nc.sync.dma_start(wc1[:], wcat_in[1])
            idt = cpool.tile([P, P], bf16, tag="idt")
            nc.sync.dma_start(idt[:], ident_in[:])
            alp = cpool.tile([2, 16], bf16, tag="alp")
            nc.sync.dma_start(alp[:], alpad_in[:])
            idx_t = cpool.tile([P, tot8], i16, tag="idx_t")
            nc.sync.dma_start(idx_t[:], idx_in[:])

            # ---- phase A: projection table
            with (
                tc.tile_pool(name="pa", bufs=3) as pa,
                tc.tile_pool(name="pa_ps", bufs=4, space="PSUM") as paps,
            ):
                for g in range(T // G8):
                    tiles = slice(g * G8, (g + 1) * G8)
                    ld0 = pa.tile([P, G8, P], f32r, tag="ld0")
                    ld1 = pa.tile([P, G8, P], f32r, tag="ld1")
                    nc.sync.dma_start(
                        ld0[:], xt_in[tiles, 0].rearrange("u p n -> p u n"))
                    nc.sync.dma_start(
                        ld1[:], xt_in[tiles, 1].rearrange("u p n -> p u n"))
                    sbX = pa.tile([P, G8, RCOL], bf16, tag="sbX")
                    for u in range(G8):
                        ps = paps.tile([P, PSROW], f32)
                        nc.tensor.matmul(ps[:], lhsT=ld0[:, u, :],
                                         rhs=wc0[:], start=True, stop=False)
                        nc.tensor.matmul(ps[:], lhsT=ld1[:, u, :],
                                         rhs=wc1[:], start=False, stop=True)
                        nc.scalar.activation(out=sbX[:, u, :], in_=ps[:],
                                             func=Act.Copy)
                    dst = table[g * G8 * P: (g + 1) * G8 * P, :].rearrange(
                        "(u p) r -> p u r", p=P)
                    nc.sync.dma_start(dst[:, :, 0:RCOL], sbX[:])
            # patch pad-row attention logits to -80
            nc.sync.dma_start(table[PADLO: PADLO + 1, HC: HC + 16],
                              alp[0:1, :])
            nc.sync.dma_start(table[PADHI: PADHI + 1, HC: HC + 16],
                              alp[1:2, :])

            # ---- phase B: per dst-block gather + attention + accumulate
            with (
                tc.tile_pool(name="gat", bufs=6) as gp,
                tc.tile_pool(name="mt", bufs=6) as mp,
                tc.tile_pool(name="small", bufs=6) as sp,
                tc.tile_pool(name="ps", bufs=2, space="PSUM") as psp,
            ):
                qrr = [0]
                off8 = [0]

                def gather(m, hi_base, tag):
                    gt = gp.tile([P, MAXJ, ROW], bf16, tag=tag)
                    src_ap = table[HIBASE:, :] if hi_base else table[:, :]
                    nc.gpsimd.dma_gather(
                        gt[:, 0:m, :], src_ap,
                        idx_t[:, off8[0]: off8[0] + m * 8],
                        m * P, m * P, ROW, queue_num=qrr[0])
                    qrr[0] = (qrr[0] + 1) % NQ
                    off8[0] += m * 8
                    return gt

                for pos in range(bpc):
                    ncalls = len(calls_pos[pos])
                    U = psp.tile([P, HC + H], f32)
                    ar_bc = None
                    ci = 0
                    for binf, m in calls_pos[pos]:
                        gt = gather(m, binf == 1, "G")
                        if ar_bc is None:
                            ar_bc = gt[:, 0:1, HC + H: HC + 2 * H]
                        lg = sp.tile([P, MAXJ, H], f32, tag="lg")
                        nc.vector.tensor_tensor(
                            out=lg[:, 0:m, :],
                            in0=gt[:, 0:m, HC: HC + H],
                            in1=ar_bc.to_broadcast([P, m, H]),
                            op=Alu.add)
                        mt = mp.tile([P, MAXJ, HC + H], bf16, tag="MT")
                        e2 = sp.tile([P, MAXJ, H], bf16, tag="e2")
                        nc.scalar.activation(out=mt[:, 0:m, HC: HC + H],
                                             in_=lg[:, 0:m, :], func=Act.Exp)
                        nc.scalar.activation(out=e2[:, 0:m, :],
                                             in_=lg[:, 0:m, :], func=Act.Exp,
                                             scale=0.2)
                        nc.vector.tensor_tensor(
                            out=mt[:, 0:m, HC: HC + H],
                            in0=mt[:, 0:m, HC: HC + H],
                            in1=e2[:, 0:m, :], op=Alu.max)
                        nc.vector.tensor_tensor(
                            out=mt[:, 0:m, 0:HC].rearrange(
                                "p m (h c) -> p m h c", c=C),
                            in0=gt[:, 0:m, 0:HC].rearrange(
                                "p m (h c) -> p m h c", c=C),
                            in1=mt[:, 0:m, HC: HC + H].unsqueeze(3)
                                .to_broadcast([P, m, H, C]),
                            op=Alu.mult)
                        for j in range(m):
                            nc.tensor.matmul(
                                U[:], lhsT=idt[:], rhs=mt[:, j, :],
                                start=(ci == 0 and j == 0),
                                stop=(ci == ncalls - 1 and j == m - 1))
                        ci += 1
                    den = sp.tile([P, H], f32, tag="den")
                    nc.vector.tensor_scalar(den[:], U[:, HC: HC + H], 1e-6,
                                            None, Alu.max)
                    rec = sp.tile([P, H], f32, tag="rec")
                    nc.vector.reciprocal(rec[:], den[:])
                    ob = sp.tile([P, HC], bf16, tag="ob")
                    nc.vector.tensor_tensor(
                        out=ob[:].rearrange("p (h c) -> p h c", c=C),
                        in0=U[:, 0:HC].rearrange("p (h c) -> p h c", c=C),
                        in1=rec[:].unsqueeze(2).to_broadcast([P, H, C]),
                        op=Alu.mult)
                    nc.sync.dma_start(out_ex[pos * P: (pos + 1) * P, :], ob[:])
    nc.compile()
    return nc


# ---------------------------------------------------------------- runner
def _run(inputs, trace=False, n_cores=8):
    from concourse.bass_utils import run_bass_kernel_spmd

    x = np.asarray(inputs["x"])
    edge_index = np.asarray(inputs["edge_index"])
    meta, shared, per_core = _preprocess(
        x, edge_index, inputs["W"], inputs["attn_l"], inputs["attn_r"], n_cores
    )
    nc = _build_program(meta)
    in_maps = [{**shared, **pc} for pc in per_core]
    res = run_bass_kernel_spmd(nc, in_maps, list(range(n_cores)), trace=trace)

    # reassemble: block at (pos, core) covers table rows [b*128, b*128+128)
    blk_at = meta["blk_at"]
    bpc, row_of = meta["bpc"], meta["row_of"]
    full = np.zeros((SLOTS, HC), np.float32)
    for c in range(n_cores):
        shard = np.asarray(res.results[c]["out"], dtype=np.float32)
        for pos in range(bpc):
            b = int(blk_at[pos, c])
            full[b * P: (b + 1) * P] = shard[pos * P: (pos + 1) * P]
    out = full[row_of]
    return np.ascontiguousarray(out), res, meta


def kernel(**inputs) -> np.ndarray:
    out, _, _ = _run(inputs, trace=False)
    return out


# revision 16
# speedup vs baseline: 1.6634x; 1.6634x over previous
"""CustomGAT (gnn_message_passing) Trainium2 kernel — 8-core SPMD, v2.

Design (dst-resident, free-axis edge layout):
  * Host: order nodes by (lo-deg, hi-deg) lexsort so consecutive 128-node
    dst blocks have near-equal per-bin in-degrees (few % slot padding).
    Edges of dst p live on partition p along the free axis (j-slots);
    per-edge src table rows split into two int16-addressable bins
    (rows < 32768, base 0; rows >= 32768, base 17408).
  * Device phase A (replicated on all cores): projection table rows
    [xp bf16 x256 | al bf16 x8 | ar bf16 x8 | pad to 768B] via fp32r
    matmuls; one ACT copy per tile PSUM->SBUF. Pad rows 0 and 50001 get
    al=ar=-80 so padded gather slots contribute exp(lrelu(-160+..)) ~ 0.
  * Device phase B (49 dst blocks per core): one m=1 "self" gather call
    brings each dst's own row (= the self-loop edge AND the block's ar
    column); real edges batch-gathered by the GPSIMD dma_gather ucode
    round-robined over 4 SWDGE queues (the descriptor-generation
    bottleneck measured 8.5 ns/row on one queue, ~3.3 ns/row on four).
    Logits al+ar on DVE; exp(leaky_relu(x)) = max(exp(x), exp(0.2x)) via
    two ACT exps + DVE max; messages xp*t on DVE; j-slot reduction via
    identity matmuls accumulating into PSUM [128, 264] whose last 8 cols
    collect the softmax denominator; one normalize per block.
  * Host: inverse-permute per-core shards to the full [N, 256] output.
"""

import numpy as np

# ---------------------------------------------------------------- constants
H = 8
C = 32
HC = H * C  # 256
IN = 256
P = 128
ROW = 384   # bf16 row: [xp 0:256 | al 256:264 | ar 264:272 | pad]
RCOL = 272  # used row columns
PSROW = 272
SLOTS = 50176  # 392 * 128; row 0 = lo pad, rows 50001+ = hi pad area
NBLK = SLOTS // P  # 392
NLO = 256   # blocks 0..255 have rows < 32768
LOWMAX = 32768
HIBASE = 17408  # hi bin rows [32768, 50176) addressed as row - 17408
PADLO = 0
PADHI = 50001
MAXJ = 8  # j-slots per dma_gather call (1024-index ucode cap)
NQ = 4    # SWDGE queues
G8 = 8    # phase A tiles per DMA group


# ---------------------------------------------------------------- tile patch
def _install_tile_patch():
    """The axon-path walrus rejects >2 sync waits on one instruction; split
    the TileContext tail-drain waits into one carrier drain per proc."""
    import concourse.tile as tile
    from concourse.vector_clock import ScopedClock, VectorClock

    if getattr(tile.TileContext, "_drain_patch_installed", False):
        return

    def _drain_and_barrier(self, tick_clock, wait_clock):
        gc = tick_clock.global_clock
        n = len(gc)
        for p in range(n):
            if gc[p] == 0:
                continue
            req = VectorClock([gc[q] if q == p else 0 for q in range(n)])
            d = self.nc.sync.drain()
            wait_clock.add_sem_waits(d.ins, ScopedClock({None: req}))
        self.nc.all_engine_barrier()
        assert self.sems is not None
        popped = self.nc._tile_sem_poison_stack.pop()
        assert popped is self._sem_poison
        self.nc.clear_and_free_semaphores(list(self.sems.allocated().values()))
        self.nc.all_engine_barrier()

    tile.TileContext._drain_and_barrier = _drain_and_barrier
    tile.TileContext._drain_patch_installed = True


# ---------------------------------------------------------------- host prep
def _idx16(vals, m):
    """Encode row indices for dma_gather: [128, m*8] int16, index k at
    [k%16 (+16*rep), k//16], replicated for the 8 Q7 cores."""
    enc = vals.astype(np.int64).astype(np.int16)
    a = enc.reshape(m * 8, 16).T  # [16, m*8]
    return np.tile(a, (8, 1))


def _calls_of(k):
    return [min(MAXJ, k - s) for s in range(0, k, MAXJ)]


def _preprocess(x, edge_index, W, attn_l, attn_r, n_cores):
    import ml_dtypes

    N = x.shape[0]
    src = np.asarray(edge_index[0]).astype(np.int64)
    dst = np.asarray(edge_index[1]).astype(np.int64)

    deg = np.bincount(dst, minlength=N)
    # membership of the lo row range is fixed up front (top in-degree nodes)
    # so each edge's bin never moves; then sort within each set by
    # (lo-deg, hi-deg) for near-uniform per-block per-bin degrees.
    S_nodes = np.argsort(-deg, kind="stable")[: LOWMAX - 1]
    inS = np.zeros(N, bool)
    inS[S_nodes] = True
    lo = np.bincount(dst[inS[src]], minlength=N)
    hi = deg - lo
    T_nodes = np.flatnonzero(~inS)
    Ssort = S_nodes[np.lexsort((hi[S_nodes], lo[S_nodes]))]
    Tsort = T_nodes[np.lexsort((hi[T_nodes], lo[T_nodes]))]
    row_of = np.empty(N, np.int64)
    row_of[Ssort] = 1 + np.arange(len(Ssort))  # row 0 reserved for lo-pad
    row_of[Tsort] = LOWMAX + np.arange(len(Tsort))

    srow = row_of[src]
    drow = row_of[dst]
    ishi = (srow >= LOWMAX).astype(np.int64)

    eblk = drow // P
    epart = drow % P
    cnt = np.zeros((NBLK, P, 2), np.int64)
    np.add.at(cnt, (eblk, epart, ishi), 1)
    klo_b = cnt[:, :, 0].max(axis=1)
    khi_b = cnt[:, :, 1].max(axis=1)
    w_b = klo_b + khi_b

    # positions: 0..31 lo-group (blocks 0..255), 32..48 hi-group.
    # 8 CONSECUTIVE lexsorted blocks share a position so their (klo, khi)
    # nearly coincide and the per-position max costs ~1 extra slot.
    bpc = NBLK // n_cores  # 49
    npos_lo = NLO // n_cores  # 32
    blk_at = np.arange(NBLK, dtype=np.int64).reshape(bpc, n_cores)

    # uniform per-position call shapes (max over the 8 cores' blocks);
    # the self row is slot j=0 of the block's own bin (lo for pos<npos_lo)
    klo_pos = klo_b[blk_at].max(axis=1)
    khi_pos = khi_b[blk_at].max(axis=1)
    klo_pos[:npos_lo] += 1
    khi_pos[npos_lo:] += 1
    tot_slots = int((klo_pos + khi_pos).sum() * P)

    # pack j-slots of consecutive positions into full MAXJ-slot calls,
    # one stream per bin (the gather base differs per bin)
    calls = []  # (binf, mcall, [(pos, j0_in_bin, off_in_call, m)])
    for binf, kvec in ((0, klo_pos), (1, khi_pos)):
        cur, fill = [], 0
        for pos in range(bpc):
            k = int(kvec[pos])
            j0 = 0
            while j0 < k:
                m = min(MAXJ - fill, k - j0)
                cur.append((pos, j0, fill, m))
                fill += m
                j0 += m
                if fill == MAXJ:
                    calls.append((binf, MAXJ, cur))
                    cur, fill = [], 0
        if fill:
            calls.append((binf, fill, cur))
    calls.sort(key=lambda cl: (min(s[0] for s in cl[2]), cl[0]))
    segs_pos = [[] for _ in range(bpc)]
    for cid, (binf, mc, segs) in enumerate(calls):
        for pos, j0, off, m in segs:
            segs_pos[pos].append((cid, off, j0, m, binf))
    for pos in range(bpc):
        selfbin = 0 if pos < npos_lo else 1
        segs_pos[pos].sort(key=lambda s: (s[4] != selfbin, s[2]))

    # per-(block, bin) gather index grids [k, P], padded by bin pad row
    idx_lo = {b: np.full((int(klo_pos[pos]), P), PADLO, np.int64)
              for pos in range(bpc) for b in blk_at[pos]}
    idx_hi = {b: np.full((int(khi_pos[pos]), P), PADHI - HIBASE, np.int64)
              for pos in range(bpc) for b in blk_at[pos]}
    # j index per edge: cumcount within (block, part, bin)
    key = (eblk * P + epart) * 2 + ishi
    eorder = np.argsort(key, kind="stable")
    ks = key[eorder]
    grp_start = np.flatnonzero(np.concatenate([[True], ks[1:] != ks[:-1]]))
    sizes = np.diff(np.concatenate([grp_start, [len(ks)]]))
    jvals = np.arange(len(ks)) - np.repeat(grp_start, sizes)
    ej = np.empty(len(ks), np.int64)
    ej[eorder] = jvals

    # edges land at j (+1 when their bin carries the self slot at j=0)
    lo_m = ishi == 0
    bs, ps, js, rs = eblk[lo_m], epart[lo_m], ej[lo_m], srow[lo_m]
    shift = (bs < NLO).astype(np.int64)
    for b in range(NBLK):
        m = bs == b
        idx_lo[b][js[m] + shift[m], ps[m]] = rs[m]
    hi_m = ishi == 1
    bs, ps, js, rs = eblk[hi_m], epart[hi_m], ej[hi_m], srow[hi_m] - HIBASE
    shift = (bs >= NLO).astype(np.int64)
    for b in range(NBLK):
        m = bs == b
        idx_hi[b][js[m] + shift[m], ps[m]] = rs[m]
    for b in range(NBLK):
        base = b * P
        selfidx = np.arange(base, base + P, dtype=np.int64)
        if b < NLO:
            idx_lo[b][0] = selfidx
        else:
            idx_hi[b][0] = selfidx - HIBASE

    # per-core concatenated idx16 in packed-call order
    core_idx = []
    for c in range(n_cores):
        chunks = []
        for binf, mc, segs in calls:
            arr = np.empty((mc, P), np.int64)
            for pos, j0, off, m in segs:
                b = int(blk_at[pos, c])
                grid = idx_lo[b] if binf == 0 else idx_hi[b]
                arr[off: off + m] = grid[j0: j0 + m]
            chunks.append(_idx16(arr.reshape(-1), mc))
        core_idx.append(np.ascontiguousarray(np.concatenate(chunks, axis=1)))
    tot8 = core_idx[0].shape[1]

    # weights: Wcat [256, 272] = [W.T | B_l | B_r]
    W = np.asarray(W, dtype=np.float32)
    attn_l = np.asarray(attn_l, dtype=np.float32).reshape(H, C)
    attn_r = np.asarray(attn_r, dtype=np.float32).reshape(H, C)
    A_l = np.zeros((HC, H), dtype=np.float32)
    A_r = np.zeros((HC, H), dtype=np.float32)
    for h in range(H):
        A_l[h * C: (h + 1) * C, h] = attn_l[h]
        A_r[h * C: (h + 1) * C, h] = attn_r[h]
    WT = np.ascontiguousarray(W.T)
    wcat = np.concatenate([WT, WT @ A_l, WT @ A_r], axis=1)  # [256, 272]
    wcat = np.ascontiguousarray(wcat.reshape(2, P, PSROW)).astype(np.float32)

    # x tiles: [T, 2, 128(in), 128(node)] so xp rows emerge in table order
    T = SLOTS // P
    x_slot = np.zeros((SLOTS, IN), dtype=np.float32)
    x_slot[row_of] = np.asarray(x, dtype=np.float32)
    xt = np.ascontiguousarray(
        x_slot.reshape(T, P, 2, P).transpose(0, 2, 3, 1), dtype=np.float32)

    alpad = np.full((2, 16), -80.0, dtype=ml_dtypes.bfloat16)
    ident = np.eye(P, dtype=ml_dtypes.bfloat16)

    meta = dict(n_cores=n_cores, T=T, bpc=bpc, row_of=row_of, blk_at=blk_at,
                calls=[(b, m) for b, m, _ in calls], segs_pos=segs_pos,
                tot8=tot8, tot_slots=tot_slots)
    shared = dict(xt=xt, wcat=wcat, alpad=alpad, ident=ident)
    per_core = [dict(idx=core_idx[c]) for c in range(n_cores)]
    return meta, shared, per_core


# ---------------------------------------------------------------- device IR
def _build_program(meta):
    import concourse.bacc as bacc
    import concourse.tile as tile
    from concourse import mybir

    _install_tile_patch()

    T, bpc, tot8 = meta["T"], meta["bpc"], meta["tot8"]
    calls, segs_pos = meta["calls"], meta["segs_pos"]
    n_cores = meta["n_cores"]
    npos_lo = NLO // n_cores
    f32 = mybir.dt.float32
    bf16 = mybir.dt.bfloat16
    i16 = mybir.dt.int16
    f32r = mybir.dt.float32r
    Alu = mybir.AluOpType
    Act = mybir.ActivationFunctionType

    nc = bacc.Bacc("TRN2", target_bir_lowering=False, debug=False,
                   num_devices=n_cores, num_swdge_queues=NQ)
    xt_in = nc.dram_tensor("xt", [T, 2, P, P], f32r, kind="ExternalInput").ap()
    wcat_in = nc.dram_tensor("wcat", [2, P, PSROW], f32r,
                             kind="ExternalInput").ap()
    alpad_in = nc.dram_tensor("alpad", [2, 16], bf16, kind="ExternalInput").ap()
    ident_in = nc.dram_tensor("ident", [P, P], bf16, kind="ExternalInput").ap()
    idx_in = nc.dram_tensor("idx", [P, tot8], i16, kind="ExternalInput").ap()
    out_ex = nc.dram_tensor("out", [bpc * P, HC], bf16,
                            kind="ExternalOutput").ap()

    with tile.TileContext(nc) as tc:
        with (
            tc.tile_pool(name="const", bufs=1) as cpool,
            tc.tile_pool(name="dram", bufs=1, space="DRAM") as dpool,
        ):
            table = dpool.tile([SLOTS, ROW], bf16)
            wc0 = cpool.tile([P, PSROW], f32r, tag="wc0")
            wc1 = cpool.tile([P, PSROW], f32r, tag="wc1")
            nc.sync.dma_start(wc0[:], wcat_in[0])
            nc.sync.dma_start(wc1[:], wcat_in[1])
            idt = cpool.tile([P, P], bf16, tag="idt")
            nc.sync.dma_start(idt[:], ident_in[:])
            alp = cpool.tile([2, 16], bf16, tag="alp")
            nc.sync.dma_start(alp[:], alpad_in[:])
            idx_t = cpool.tile([P, tot8], i16, tag="idx_t")
            nc.sync.dma_start(idx_t[:], idx_in[:])

            # ---- phase A: projection table
            with (
                tc.tile_pool(name="pa", bufs=3) as pa,
                tc.tile_pool(name="pa_ps", bufs=4, space="PSUM") as paps,
            ):
                for g in range(T // G8):
                    tiles = slice(g * G8, (g + 1) * G8)
                    ld0 = pa.tile([P, G8, P], f32r, tag="ld0")
                    ld1 = pa.tile([P, G8, P], f32r, tag="ld1")
                    nc.sync.dma_start(
                        ld0[:], xt_in[tiles, 0].rearrange("u p n -> p u n"))
                    nc.sync.dma_start(
                        ld1[:], xt_in[tiles, 1].rearrange("u p n -> p u n"))
                    sbX = pa.tile([P, G8, RCOL], bf16, tag="sbX")
                    for u in range(G8):
                        ps = paps.tile([P, PSROW], f32)
                        nc.tensor.matmul(ps[:], lhsT=ld0[:, u, :],
                                         rhs=wc0[:], start=True, stop=False)
                        nc.tensor.matmul(ps[:], lhsT=ld1[:, u, :],
                                         rhs=wc1[:], start=False, stop=True)
                        nc.scalar.activation(out=sbX[:, u, :], in_=ps[:],
                                             func=Act.Copy)
                    dst = table[g * G8 * P: (g + 1) * G8 * P, :].rearrange(
                        "(u p) r -> p u r", p=P)
                    nc.sync.dma_start(dst[:, :, 0:RCOL], sbX[:])
            # patch pad-row attention logits to -80
            nc.sync.dma_start(table[PADLO: PADLO + 1, HC: HC + 16],
                              alp[0:1, :])
            nc.sync.dma_start(table[PADHI: PADHI + 1, HC: HC + 16],
                              alp[1:2, :])

            # ---- phase B: per dst-block gather + attention + accumulate
            with (
                tc.tile_pool(name="gat", bufs=10) as gp,
                tc.tile_pool(name="mt", bufs=6) as mp,
                tc.tile_pool(name="small", bufs=6) as sp,
                tc.tile_pool(name="ps", bufs=2, space="PSUM") as psp,
            ):
                qrr = [0]
                off8 = [0]
                call_tiles = {}
                next_call = [0]

                def gather_next():
                    cid = next_call[0]
                    binf, mc = calls[cid]
                    gt = gp.tile([P, MAXJ, ROW], bf16, tag="G")
                    src_ap = table[HIBASE:, :] if binf else table[:, :]
                    nc.gpsimd.dma_gather(
                        gt[:, 0:mc, :], src_ap,
                        idx_t[:, off8[0]: off8[0] + mc * 8],
                        mc * P, mc * P, ROW, queue_num=qrr[0])
                    qrr[0] = (qrr[0] + 1) % NQ
                    off8[0] += mc * 8
                    call_tiles[cid] = gt
                    next_call[0] += 1

                for pos in range(bpc):
                    segs = segs_pos[pos]
                    needed = max(s[0] for s in segs)
                    while next_call[0] <= needed:
                        gather_next()
                    U = psp.tile([P, HC + H], f32)
                    cid0, off0 = segs[0][0], segs[0][1]
                    ar_bc = call_tiles[cid0][:, off0: off0 + 1,
                                             HC + H: HC + 2 * H]
                    totj = sum(s[3] for s in segs)
                    ji = 0
                    for cid, off, j0, m, binf in segs:
                        gt = call_tiles[cid]
                        sl = slice(off, off + m)
                        lg = sp.tile([P, MAXJ, H], f32, tag="lg")
                        nc.vector.tensor_tensor(
                            out=lg[:, 0:m, :],
                            in0=gt[:, sl, HC: HC + H],
                            in1=ar_bc.to_broadcast([P, m, H]),
                            op=Alu.add)
                        mt = mp.tile([P, MAXJ, HC + H], bf16, tag="MT")
                        e2 = sp.tile([P, MAXJ, H], bf16, tag="e2")
                        nc.scalar.activation(out=mt[:, 0:m, HC: HC + H],
                                             in_=lg[:, 0:m, :], func=Act.Exp)
                        nc.scalar.activation(out=e2[:, 0:m, :],
                                             in_=lg[:, 0:m, :], func=Act.Exp,
                                             scale=0.2)
                        nc.vector.tensor_tensor(
                            out=mt[:, 0:m, HC: HC + H],
                            in0=mt[:, 0:m, HC: HC + H],
                            in1=e2[:, 0:m, :], op=Alu.max)
                        nc.vector.tensor_tensor(
                            out=mt[:, 0:m, 0:HC].rearrange(
                                "p m (h c) -> p m h c", c=C),
                            in0=gt[:, sl, 0:HC].rearrange(
                                "p m (h c) -> p m h c", c=C),
                            in1=mt[:, 0:m, HC: HC + H].unsqueeze(3)
                                .to_broadcast([P, m, H, C]),
                            op=Alu.mult)
                        for j in range(m):
                            nc.tensor.matmul(
                                U[:], lhsT=idt[:], rhs=mt[:, j, :],
                                start=(ji == 0),
                                stop=(ji == totj - 1))
                            ji += 1
                    den = sp.tile([P, H], f32, tag="den")
                    nc.vector.tensor_scalar(den[:], U[:, HC: HC + H], 1e-6,
                                            None, Alu.max)
                    rec = sp.tile([P, H], f32, tag="rec")
                    nc.vector.reciprocal(rec[:], den[:])
                    ob = sp.tile([P, HC], bf16, tag="ob")
                    nc.vector.tensor_tensor(
                        out=ob[:].rearrange("p (h c) -> p h c", c=C),
                        in0=U[:, 0:HC].rearrange("p (h c) -> p h c", c=C),
                        in1=rec[:].unsqueeze(2).to_broadcast([P, H, C]),
                        op=Alu.mult)
                    nc.sync.dma_start(out_ex[pos * P: (pos + 1) * P, :], ob[:])
    nc.compile()
    return nc


# ---------------------------------------------------------------- runner
def _run(inputs, trace=False, n_cores=8):
    from concourse.bass_utils import run_bass_kernel_spmd

    x = np.asarray(inputs["x"])
    edge_index = np.asarray(inputs["edge_index"])
    meta, shared, per_core = _preprocess(
        x, edge_index, inputs["W"], inputs["attn_l"], inputs["attn_r"], n_cores
    )
    nc = _build_program(meta)
    in_maps = [{**shared, **pc} for pc in per_core]
    res = run_bass_kernel_spmd(nc, in_maps, list(range(n_cores)), trace=trace)

    # reassemble: block at (pos, core) covers table rows [b*128, b*128+128)
    blk_at = meta["blk_at"]
    bpc, row_of = meta["bpc"], meta["row_of"]
    full = np.zeros((SLOTS, HC), np.float32)
    for c in range(n_cores):
        shard = np.asarray(res.results[c]["out"], dtype=np.float32)
        for pos in range(bpc):
            b = int(blk_at[pos, c])
            full[b * P: (b + 1) * P] = shard[pos * P: (pos + 1) * P]
    out = full[row_of]
    return np.ascontiguousarray(out), res, meta


def kernel(**inputs) -> np.ndarray:
    out, _, _ = _run(inputs, trace=False)
    return out


# revision 23
# speedup vs baseline: 1.9546x; 1.1751x over previous
"""CustomGAT (gnn_message_passing) Trainium2 kernel — 8-core SPMD, v2.

Design (dst-resident, free-axis edge layout):
  * Host: order nodes by (lo-deg, hi-deg) lexsort so consecutive 128-node
    dst blocks have near-equal per-bin in-degrees (few % slot padding).
    Edges of dst p live on partition p along the free axis (j-slots);
    per-edge src table rows split into two int16-addressable bins
    (rows < 32768, base 0; rows >= 32768, base 17408).
  * Device phase A (replicated on all cores): projection table rows
    [xp bf16 x256 | al bf16 x8 | ar bf16 x8 | pad to 768B] via fp32r
    matmuls; one ACT copy per tile PSUM->SBUF. Pad rows 0 and 50001 get
    al=ar=-80 so padded gather slots contribute exp(lrelu(-160+..)) ~ 0.
  * Device phase B (49 dst blocks per core): one m=1 "self" gather call
    brings each dst's own row (= the self-loop edge AND the block's ar
    column); real edges batch-gathered by the GPSIMD dma_gather ucode
    round-robined over 4 SWDGE queues (the descriptor-generation
    bottleneck measured 8.5 ns/row on one queue, ~3.3 ns/row on four).
    Logits al+ar on DVE; exp(leaky_relu(x)) = max(exp(x), exp(0.2x)) via
    two ACT exps + DVE max; messages xp*t on DVE; j-slot reduction via
    identity matmuls accumulating into PSUM [128, 264] whose last 8 cols
    collect the softmax denominator; one normalize per block.
  * Host: inverse-permute per-core shards to the full [N, 256] output.
"""

import numpy as np

# ---------------------------------------------------------------- constants
H = 8
C = 32
HC = H * C  # 256
IN = 256
P = 128
ROW = 384   # bf16 row: [xp 0:256 | al 256:264 | ar 264:272 | pad]
RCOL = 272  # used row columns
PSROW = 272
SLOTS = 50176  # 392 * 128; row 0 = lo pad, rows 50001+ = hi pad area
NBLK = SLOTS // P  # 392
NLO = 256   # blocks 0..255 have rows < 32768
LOWMAX = 32768
HIBASE = 17408  # hi bin rows [32768, 50176) addressed as row - 17408
PADLO = 0
PADHI = 50001
MAXJ = 8  # j-slots per dma_gather call (1024-index ucode cap)
NQ = 4    # SWDGE queues
G8 = 8    # phase A tiles per DMA group


# ---------------------------------------------------------------- tile patch
def _install_tile_patch():
    """The axon-path walrus rejects >2 sync waits on one instruction; split
    the TileContext tail-drain waits into one carrier drain per proc."""
    import concourse.tile as tile
    from concourse.vector_clock import ScopedClock, VectorClock

    if getattr(tile.TileContext, "_drain_patch_installed", False):
        return

    def _drain_and_barrier(self, tick_clock, wait_clock):
        gc = tick_clock.global_clock
        n = len(gc)
        for p in range(n):
            if gc[p] == 0:
                continue
            req = VectorClock([gc[q] if q == p else 0 for q in range(n)])
            d = self.nc.sync.drain()
            wait_clock.add_sem_waits(d.ins, ScopedClock({None: req}))
        self.nc.all_engine_barrier()
        assert self.sems is not None
        popped = self.nc._tile_sem_poison_stack.pop()
        assert popped is self._sem_poison
        self.nc.clear_and_free_semaphores(list(self.sems.allocated().values()))
        self.nc.all_engine_barrier()

    tile.TileContext._drain_and_barrier = _drain_and_barrier
    tile.TileContext._drain_patch_installed = True


# ---------------------------------------------------------------- host prep
def _idx16(vals, m):
    """Encode row indices for dma_gather: [128, m*8] int16, index k at
    [k%16 (+16*rep), k//16], replicated for the 8 Q7 cores."""
    enc = vals.astype(np.int64).astype(np.int16)
    a = enc.reshape(m * 8, 16).T  # [16, m*8]
    return np.tile(a, (8, 1))


def _calls_of(k):
    return [min(MAXJ, k - s) for s in range(0, k, MAXJ)]


def _preprocess(x, edge_index, W, attn_l, attn_r, n_cores):
    import ml_dtypes

    N = x.shape[0]
    src = np.asarray(edge_index[0]).astype(np.int64)
    dst = np.asarray(edge_index[1]).astype(np.int64)

    deg = np.bincount(dst, minlength=N)
    # membership of the lo row range is fixed up front (top in-degree nodes)
    # so each edge's bin never moves; then sort within each set by
    # (lo-deg, hi-deg) for near-uniform per-block per-bin degrees.
    S_nodes = np.argsort(-deg, kind="stable")[: LOWMAX - 1]
    inS = np.zeros(N, bool)
    inS[S_nodes] = True
    lo = np.bincount(dst[inS[src]], minlength=N)
    hi = deg - lo
    T_nodes = np.flatnonzero(~inS)
    Ssort = S_nodes[np.lexsort((hi[S_nodes], lo[S_nodes]))]
    Tsort = T_nodes[np.lexsort((hi[T_nodes], lo[T_nodes]))]
    row_of = np.empty(N, np.int64)
    row_of[Ssort] = 1 + np.arange(len(Ssort))  # row 0 reserved for lo-pad
    row_of[Tsort] = LOWMAX + np.arange(len(Tsort))

    srow = row_of[src]
    drow = row_of[dst]
    ishi = (srow >= LOWMAX).astype(np.int64)

    eblk = drow // P
    epart = drow % P
    cnt = np.zeros((NBLK, P, 2), np.int64)
    np.add.at(cnt, (eblk, epart, ishi), 1)
    klo_b = cnt[:, :, 0].max(axis=1)
    khi_b = cnt[:, :, 1].max(axis=1)
    w_b = klo_b + khi_b

    # positions: 0..31 lo-group (blocks 0..255), 32..48 hi-group.
    # 8 CONSECUTIVE lexsorted blocks share a position so their (klo, khi)
    # nearly coincide and the per-position max costs ~1 extra slot.
    bpc = NBLK // n_cores  # 49
    npos_lo = NLO // n_cores  # 32
    blk_at = np.arange(NBLK, dtype=np.int64).reshape(bpc, n_cores)

    # uniform per-position call shapes (max over the 8 cores' blocks);
    # the self row is slot j=0 of the block's own bin (lo for pos<npos_lo)
    klo_pos = klo_b[blk_at].max(axis=1)
    khi_pos = khi_b[blk_at].max(axis=1)
    klo_pos[:npos_lo] += 1
    khi_pos[npos_lo:] += 1
    tot_slots = int((klo_pos + khi_pos).sum() * P)

    # pack j-slots of consecutive positions into full MAXJ-slot calls,
    # one stream per bin (the gather base differs per bin)
    calls = []  # (binf, mcall, [(pos, j0_in_bin, off_in_call, m)])
    for binf, kvec in ((0, klo_pos), (1, khi_pos)):
        cur, fill = [], 0
        for pos in range(bpc):
            k = int(kvec[pos])
            j0 = 0
            while j0 < k:
                m = min(MAXJ - fill, k - j0)
                cur.append((pos, j0, fill, m))
                fill += m
                j0 += m
                if fill == MAXJ:
                    calls.append((binf, MAXJ, cur))
                    cur, fill = [], 0
        if fill:
            calls.append((binf, fill, cur))
    calls.sort(key=lambda cl: (min(s[0] for s in cl[2]), cl[0]))
    segs_pos = [[] for _ in range(bpc)]
    for cid, (binf, mc, segs) in enumerate(calls):
        for pos, j0, off, m in segs:
            segs_pos[pos].append((cid, off, j0, m, binf))
    for pos in range(bpc):
        selfbin = 0 if pos < npos_lo else 1
        segs_pos[pos].sort(key=lambda s: (s[4] != selfbin, s[2]))

    # per-(block, bin) gather index grids [k, P], padded by bin pad row
    idx_lo = {b: np.full((int(klo_pos[pos]), P), PADLO, np.int64)
              for pos in range(bpc) for b in blk_at[pos]}
    idx_hi = {b: np.full((int(khi_pos[pos]), P), PADHI - HIBASE, np.int64)
              for pos in range(bpc) for b in blk_at[pos]}
    # j index per edge: cumcount within (block, part, bin)
    key = (eblk * P + epart) * 2 + ishi
    eorder = np.argsort(key, kind="stable")
    ks = key[eorder]
    grp_start = np.flatnonzero(np.concatenate([[True], ks[1:] != ks[:-1]]))
    sizes = np.diff(np.concatenate([grp_start, [len(ks)]]))
    jvals = np.arange(len(ks)) - np.repeat(grp_start, sizes)
    ej = np.empty(len(ks), np.int64)
    ej[eorder] = jvals

    # edges land at j (+1 when their bin carries the self slot at j=0)
    lo_m = ishi == 0
    bs, ps, js, rs = eblk[lo_m], epart[lo_m], ej[lo_m], srow[lo_m]
    shift = (bs < NLO).astype(np.int64)
    for b in range(NBLK):
        m = bs == b
        idx_lo[b][js[m] + shift[m], ps[m]] = rs[m]
    hi_m = ishi == 1
    bs, ps, js, rs = eblk[hi_m], epart[hi_m], ej[hi_m], srow[hi_m] - HIBASE
    shift = (bs >= NLO).astype(np.int64)
    for b in range(NBLK):
        m = bs == b
        idx_hi[b][js[m] + shift[m], ps[m]] = rs[m]
    for b in range(NBLK):
        base = b * P
        selfidx = np.arange(base, base + P, dtype=np.int64)
        if b < NLO:
            idx_lo[b][0] = selfidx
        else:
            idx_hi[b][0] = selfidx - HIBASE

    # per-core concatenated idx16 in packed-call order
    core_idx = []
    for c in range(n_cores):
        chunks = []
        for binf, mc, segs in calls:
            arr = np.empty((mc, P), np.int64)
            for pos, j0, off, m in segs:
                b = int(blk_at[pos, c])
                grid = idx_lo[b] if binf == 0 else idx_hi[b]
                arr[off: off + m] = grid[j0: j0 + m]
            chunks.append(_idx16(arr.reshape(-1), mc))
        core_idx.append(np.ascontiguousarray(np.concatenate(chunks, axis=1)))
    tot8 = core_idx[0].shape[1]

    # weights: Wcat [256, 272] = [W.T | B_l | B_r]
    W = np.asarray(W, dtype=np.float32)
    attn_l = np.asarray(attn_l, dtype=np.float32).reshape(H, C)
    attn_r = np.asarray(attn_r, dtype=np.float32).reshape(H, C)
    A_l = np.zeros((HC, H), dtype=np.float32)
    A_r = np.zeros((HC, H), dtype=np.float32)
    for h in range(H):
        A_l[h * C: (h + 1) * C, h] = attn_l[h]
        A_r[h * C: (h + 1) * C, h] = attn_r[h]
    WT = np.ascontiguousarray(W.T)
    wcat = np.concatenate([WT, WT @ A_l, WT @ A_r], axis=1)  # [256, 272]
    wcat = np.ascontiguousarray(wcat.reshape(2, P, PSROW)).astype(np.float32)

    # x tiles, pre-permuted so the phase-A lhsT loads are contiguous:
    # xt[g, h, p, u*128+n] = x_slot[g*1024 + u*128 + n, h*128 + p]
    T = SLOTS // P
    x_slot = np.zeros((SLOTS, IN), dtype=np.float32)
    x_slot[row_of] = np.asarray(x, dtype=np.float32)
    xt = np.ascontiguousarray(
        x_slot.reshape(T // G8, G8, P, 2, P).transpose(0, 3, 4, 1, 2)
        .reshape(T // G8, 2, P, G8 * P), dtype=np.float32)

    alpad = np.full((2, 16), -80.0, dtype=ml_dtypes.bfloat16)
    ident = np.eye(P, dtype=ml_dtypes.bfloat16)

    meta = dict(n_cores=n_cores, T=T, bpc=bpc, row_of=row_of, blk_at=blk_at,
                calls=[(b, m) for b, m, _ in calls], segs_pos=segs_pos,
                tot8=tot8, tot_slots=tot_slots)
    shared = dict(xt=xt, wcat=wcat, alpad=alpad, ident=ident)
    per_core = [dict(idx=core_idx[c]) for c in range(n_cores)]
    return meta, shared, per_core


# ---------------------------------------------------------------- device IR
def _build_program(meta):
    import concourse.bacc as bacc
    import concourse.tile as tile
    from concourse import mybir

    _install_tile_patch()

    T, bpc, tot8 = meta["T"], meta["bpc"], meta["tot8"]
    calls, segs_pos = meta["calls"], meta["segs_pos"]
    n_cores = meta["n_cores"]
    npos_lo = NLO // n_cores
    f32 = mybir.dt.float32
    bf16 = mybir.dt.bfloat16
    i16 = mybir.dt.int16
    f32r = mybir.dt.float32r
    Alu = mybir.AluOpType
    Act = mybir.ActivationFunctionType

    nc = bacc.Bacc("TRN2", target_bir_lowering=False, debug=False,
                   num_devices=n_cores, num_swdge_queues=NQ)
    xt_in = nc.dram_tensor("xt", [T // G8, 2, P, G8 * P], f32r,
                           kind="ExternalInput").ap()
    wcat_in = nc.dram_tensor("wcat", [2, P, PSROW], f32r,
                             kind="ExternalInput").ap()
    alpad_in = nc.dram_tensor("alpad", [2, 16], bf16, kind="ExternalInput").ap()
    ident_in = nc.dram_tensor("ident", [P, P], bf16, kind="ExternalInput").ap()
    idx_in = nc.dram_tensor("idx", [P, tot8], i16, kind="ExternalInput").ap()
    out_ex = nc.dram_tensor("out", [bpc * P, HC], bf16,
                            kind="ExternalOutput").ap()

    with tile.TileContext(nc) as tc:
        with (
            tc.tile_pool(name="const", bufs=1) as cpool,
            tc.tile_pool(name="dram", bufs=1, space="DRAM") as dpool,
        ):
            table = dpool.tile([SLOTS, ROW], bf16)
            wc0 = cpool.tile([P, PSROW], f32r, tag="wc0")
            wc1 = cpool.tile([P, PSROW], f32r, tag="wc1")
            nc.sync.dma_start(wc0[:], wcat_in[0])
            nc.sync.dma_start(wc1[:], wcat_in[1])
            idt = cpool.tile([P, P], bf16, tag="idt")
            nc.sync.dma_start(idt[:], ident_in[:])
            alp = cpool.tile([2, 16], bf16, tag="alp")
            nc.sync.dma_start(alp[:], alpad_in[:])
            idx_t = cpool.tile([P, tot8], i16, tag="idx_t")
            nc.sync.dma_start(idx_t[:], idx_in[:])

            # ---- phase A: projection table
            with (
                tc.tile_pool(name="pa", bufs=3) as pa,
                tc.tile_pool(name="pa_ps", bufs=4, space="PSUM") as paps,
            ):
                for g in range(T // G8):
                    ld0 = pa.tile([P, G8, P], f32r, tag="ld0")
                    ld1 = pa.tile([P, G8, P], f32r, tag="ld1")
                    nc.sync.dma_start(
                        ld0[:].rearrange("p u n -> p (u n)"), xt_in[g, 0])
                    nc.sync.dma_start(
                        ld1[:].rearrange("p u n -> p (u n)"), xt_in[g, 1])
                    sbX = pa.tile([P, G8, RCOL], bf16, tag="sbX")
                    for u in range(G8):
                        ps = paps.tile([P, PSROW], f32)
                        nc.tensor.matmul(ps[:], lhsT=ld0[:, u, :],
                                         rhs=wc0[:], start=True, stop=False)
                        nc.tensor.matmul(ps[:], lhsT=ld1[:, u, :],
                                         rhs=wc1[:], start=False, stop=True)
                        nc.scalar.activation(out=sbX[:, u, :], in_=ps[:],
                                             func=Act.Copy)
                    dst = table[g * G8 * P: (g + 1) * G8 * P, :].rearrange(
                        "(u p) r -> p u r", p=P)
                    nc.sync.dma_start(dst[:, :, 0:RCOL], sbX[:])
            # patch pad-row attention logits to -80
            nc.sync.dma_start(table[PADLO: PADLO + 1, HC: HC + 16],
                              alp[0:1, :])
            nc.sync.dma_start(table[PADHI: PADHI + 1, HC: HC + 16],
                              alp[1:2, :])

            # ---- phase B: per dst-block gather + attention + accumulate
            with (
                tc.tile_pool(name="gat", bufs=10) as gp,
                tc.tile_pool(name="mt", bufs=6) as mp,
                tc.tile_pool(name="small", bufs=6) as sp,
                tc.tile_pool(name="ps", bufs=2, space="PSUM") as psp,
            ):
                qrr = [0]
                off8 = [0]
                call_tiles = {}
                next_call = [0]

                def gather_next():
                    cid = next_call[0]
                    binf, mc = calls[cid]
                    gt = gp.tile([P, MAXJ, ROW], bf16, tag="G")
                    src_ap = table[HIBASE:, :] if binf else table[:, :]
                    nc.gpsimd.dma_gather(
                        gt[:, 0:mc, :], src_ap,
                        idx_t[:, off8[0]: off8[0] + mc * 8],
                        mc * P, mc * P, ROW, queue_num=qrr[0])
                    qrr[0] = (qrr[0] + 1) % NQ
                    off8[0] += mc * 8
                    call_tiles[cid] = gt
                    next_call[0] += 1

                for pos in range(bpc):
                    segs = segs_pos[pos]
                    needed = max(s[0] for s in segs)
                    while next_call[0] <= needed:
                        gather_next()
                    U2 = psp.tile([P, 2 * HC], f32)
                    den = sp.tile([P, H], f32, tag="den")
                    cid0, off0 = segs[0][0], segs[0][1]
                    ar_bc = call_tiles[cid0][:, off0: off0 + 1,
                                             HC + H: HC + 2 * H]
                    mmops = []  # (mt_tile, jj, width)
                    first_seg = True
                    for cid, off, j0, m, binf in segs:
                        gt = call_tiles[cid]
                        sl = slice(off, off + m)
                        lg = sp.tile([P, MAXJ, H], f32, tag="lg")
                        nc.vector.tensor_tensor(
                            out=lg[:, 0:m, :],
                            in0=gt[:, sl, HC: HC + H],
                            in1=ar_bc.to_broadcast([P, m, H]),
                            op=Alu.add)
                        mt = mp.tile([P, MAXJ, HC + H], bf16, tag="MT")
                        e2 = sp.tile([P, MAXJ, H], bf16, tag="e2")
                        nc.scalar.activation(out=mt[:, 0:m, HC: HC + H],
                                             in_=lg[:, 0:m, :], func=Act.Exp)
                        nc.scalar.activation(out=e2[:, 0:m, :],
                                             in_=lg[:, 0:m, :], func=Act.Exp,
                                             scale=0.2)
                        nc.vector.tensor_tensor(
                            out=mt[:, 0:m, HC: HC + H],
                            in0=mt[:, 0:m, HC: HC + H],
                            in1=e2[:, 0:m, :], op=Alu.max)
                        nc.vector.tensor_tensor(
                            out=mt[:, 0:m, 0:HC].rearrange(
                                "p m (h c) -> p m h c", c=C),
                            in0=gt[:, sl, 0:HC].rearrange(
                                "p m (h c) -> p m h c", c=C),
                            in1=mt[:, 0:m, HC: HC + H].unsqueeze(3)
                                .to_broadcast([P, m, H, C]),
                            op=Alu.mult)
                        # denominator on DVE: reduce t over j, accumulate
                        dseg = sp.tile([P, H], f32, tag="dseg")
                        nc.vector.tensor_reduce(
                            dseg[:],
                            mt[:, 0:m, HC: HC + H].rearrange("p m h -> p h m"),
                            mybir.AxisListType.X, Alu.add)
                        if first_seg:
                            nc.vector.tensor_scalar(den[:], dseg[:], 1e-6,
                                                    None, Alu.max)
                            first_seg = False
                        else:
                            nc.vector.tensor_tensor(out=den[:], in0=den[:],
                                                    in1=dseg[:], op=Alu.add)
                        for jj in range(0, m, 2):
                            w = 2 if jj + 1 < m else 1
                            mmops.append((mt, jj, w))
                    k2 = next((i for i, o in enumerate(mmops) if o[2] == 2), 0)
                    if k2:
                        mmops[0], mmops[k2] = mmops[k2], mmops[0]
                    haspair = mmops[0][2] == 2
                    for i, (mt, jj, w) in enumerate(mmops):
                        nc.tensor.matmul(
                            U2[:, 0: w * HC],
                            lhsT=idt[:],
                            rhs=mt[:, jj: jj + w, 0:HC],
                            start=(i == 0), stop=(i == len(mmops) - 1),
                            skip_group_check=True)
                    rec = sp.tile([P, H], f32, tag="rec")
                    nc.vector.reciprocal(rec[:], den[:])
                    us = sp.tile([P, HC], f32, tag="us")
                    if haspair:
                        u1 = sp.tile([P, HC], f32, tag="u1")
                        nc.scalar.activation(out=u1[:], in_=U2[:, HC: 2 * HC],
                                             func=Act.Copy)
                        nc.vector.tensor_tensor(out=us[:], in0=u1[:],
                                                in1=U2[:, 0:HC], op=Alu.add)
                    else:
                        nc.vector.tensor_scalar(us[:], U2[:, 0:HC], 1.0,
                                                None, Alu.mult)
                    ob = sp.tile([P, HC], bf16, tag="ob")
                    nc.vector.tensor_tensor(
                        out=ob[:].rearrange("p (h c) -> p h c", c=C),
                        in0=us[:].rearrange("p (h c) -> p h c", c=C),
                        in1=rec[:].unsqueeze(2).to_broadcast([P, H, C]),
                        op=Alu.mult)
                    nc.sync.dma_start(out_ex[pos * P: (pos + 1) * P, :], ob[:])
    nc.compile()
    return nc


# ---------------------------------------------------------------- runner
def _run(inputs, trace=False, n_cores=8):
    from concourse.bass_utils import run_bass_kernel_spmd

    x = np.asarray(inputs["x"])
    edge_index = np.asarray(inputs["edge_index"])
    meta, shared, per_core = _preprocess(
        x, edge_index, inputs["W"], inputs["attn_l"], inputs["attn_r"], n_cores
    )
    nc = _build_program(meta)
    in_maps = [{**shared, **pc} for pc in per_core]
    res = run_bass_kernel_spmd(nc, in_maps, list(range(n_cores)), trace=trace)

    # reassemble: block at (pos, core) covers table rows [b*128, b*128+128)
    blk_at = meta["blk_at"]
    bpc, row_of = meta["bpc"], meta["row_of"]
    full = np.zeros((SLOTS, HC), np.float32)
    for c in range(n_cores):
        shard = np.asarray(res.results[c]["out"], dtype=np.float32)
        for pos in range(bpc):
            b = int(blk_at[pos, c])
            full[b * P: (b + 1) * P] = shard[pos * P: (pos + 1) * P]
    out = full[row_of]
    return np.ascontiguousarray(out), res, meta


def kernel(**inputs) -> np.ndarray:
    out, _, _ = _run(inputs, trace=False)
    return out


# revision 30
# speedup vs baseline: 2.1998x; 1.1254x over previous
"""CustomGAT (gnn_message_passing) Trainium2 kernel — 8-core SPMD, v2.

Design (dst-resident, free-axis edge layout):
  * Host: order nodes by (lo-deg, hi-deg) lexsort so consecutive 128-node
    dst blocks have near-equal per-bin in-degrees (few % slot padding).
    Edges of dst p live on partition p along the free axis (j-slots);
    per-edge src table rows split into two int16-addressable bins
    (rows < 32768, base 0; rows >= 32768, base 17408).
  * Device phase A (replicated on all cores): projection table rows
    [xp bf16 x256 | al bf16 x8 | ar bf16 x8 | pad to 768B] via fp32r
    matmuls; one ACT copy per tile PSUM->SBUF. Pad rows 0 and 50001 get
    al=ar=-80 so padded gather slots contribute exp(lrelu(-160+..)) ~ 0.
  * Device phase B (49 dst blocks per core): one m=1 "self" gather call
    brings each dst's own row (= the self-loop edge AND the block's ar
    column); real edges batch-gathered by the GPSIMD dma_gather ucode
    round-robined over 4 SWDGE queues (the descriptor-generation
    bottleneck measured 8.5 ns/row on one queue, ~3.3 ns/row on four).
    Logits al+ar on DVE; exp(leaky_relu(x)) = max(exp(x), exp(0.2x)) via
    two ACT exps + DVE max; messages xp*t on DVE; j-slot reduction via
    identity matmuls accumulating into PSUM [128, 264] whose last 8 cols
    collect the softmax denominator; one normalize per block.
  * Host: inverse-permute per-core shards to the full [N, 256] output.
"""

import numpy as np

# ---------------------------------------------------------------- constants
H = 8
C = 32
HC = H * C  # 256
IN = 256
P = 128
ROW = 384   # bf16 row: [xp 0:256 | al 256:264 | ar 264:272 | pad]
RCOL = 272  # used row columns
PSROW = 272
SLOTS = 50176  # 392 * 128; row 0 = lo pad, rows 50001+ = hi pad area
NBLK = SLOTS // P  # 392
NLO = 256   # blocks 0..255 have rows < 32768
LOWMAX = 32768
HIBASE = 17408  # hi bin rows [32768, 50176) addressed as row - 17408
PADLO = 0
PADHI = 50001
MAXJ = 8  # j-slots per dma_gather call (1024-index ucode cap)
NQ = 4    # SWDGE queues
G8 = 8    # phase A tiles per DMA group


# ---------------------------------------------------------------- tile patch
def _install_tile_patch():
    """The axon-path walrus rejects >2 sync waits on one instruction; split
    the TileContext tail-drain waits into one carrier drain per proc."""
    import concourse.tile as tile
    from concourse.vector_clock import ScopedClock, VectorClock

    if getattr(tile.TileContext, "_drain_patch_installed", False):
        return

    def _drain_and_barrier(self, tick_clock, wait_clock):
        gc = tick_clock.global_clock
        n = len(gc)
        for p in range(n):
            if gc[p] == 0:
                continue
            req = VectorClock([gc[q] if q == p else 0 for q in range(n)])
            d = self.nc.sync.drain()
            wait_clock.add_sem_waits(d.ins, ScopedClock({None: req}))
        self.nc.all_engine_barrier()
        assert self.sems is not None
        popped = self.nc._tile_sem_poison_stack.pop()
        assert popped is self._sem_poison
        self.nc.clear_and_free_semaphores(list(self.sems.allocated().values()))
        self.nc.all_engine_barrier()

    tile.TileContext._drain_and_barrier = _drain_and_barrier
    tile.TileContext._drain_patch_installed = True


# ---------------------------------------------------------------- host prep
def _idx16(vals, m):
    """Encode row indices for dma_gather: [128, m*8] int16, index k at
    [k%16 (+16*rep), k//16], replicated for the 8 Q7 cores."""
    enc = vals.astype(np.int64).astype(np.int16)
    a = enc.reshape(m * 8, 16).T  # [16, m*8]
    return np.tile(a, (8, 1))


def _calls_of(k):
    return [min(MAXJ, k - s) for s in range(0, k, MAXJ)]


def _preprocess(x, edge_index, W, attn_l, attn_r, n_cores):
    import ml_dtypes

    N = x.shape[0]
    src = np.asarray(edge_index[0]).astype(np.int64)
    dst = np.asarray(edge_index[1]).astype(np.int64)

    deg = np.bincount(dst, minlength=N)
    # membership of the lo row range is fixed up front (top in-degree nodes)
    # so each edge's bin never moves; then sort within each set by
    # (lo-deg, hi-deg) for near-uniform per-block per-bin degrees.
    S_nodes = np.argsort(-deg, kind="stable")[: LOWMAX - 1]
    inS = np.zeros(N, bool)
    inS[S_nodes] = True
    lo = np.bincount(dst[inS[src]], minlength=N)
    hi = deg - lo
    T_nodes = np.flatnonzero(~inS)
    Ssort = S_nodes[np.lexsort((hi[S_nodes], lo[S_nodes]))]
    Tsort = T_nodes[np.lexsort((hi[T_nodes], lo[T_nodes]))]
    row_of = np.empty(N, np.int64)
    row_of[Ssort] = 1 + np.arange(len(Ssort))  # row 0 reserved for lo-pad
    row_of[Tsort] = LOWMAX + np.arange(len(Tsort))

    srow = row_of[src]
    drow = row_of[dst]
    ishi = (srow >= LOWMAX).astype(np.int64)

    eblk = drow // P
    epart = drow % P
    cnt = np.zeros((NBLK, P, 2), np.int64)
    np.add.at(cnt, (eblk, epart, ishi), 1)
    klo_b = cnt[:, :, 0].max(axis=1)
    khi_b = cnt[:, :, 1].max(axis=1)
    w_b = klo_b + khi_b

    # positions: 0..31 lo-group (blocks 0..255), 32..48 hi-group.
    # 8 CONSECUTIVE lexsorted blocks share a position so their (klo, khi)
    # nearly coincide and the per-position max costs ~1 extra slot.
    bpc = NBLK // n_cores  # 49
    npos_lo = NLO // n_cores  # 32
    blk_at = np.arange(NBLK, dtype=np.int64).reshape(bpc, n_cores)

    # uniform per-position call shapes (max over the 8 cores' blocks);
    # the self row is slot j=0 of the block's own bin (lo for pos<npos_lo)
    klo_pos = klo_b[blk_at].max(axis=1)
    khi_pos = khi_b[blk_at].max(axis=1)
    klo_pos[:npos_lo] += 1
    khi_pos[npos_lo:] += 1
    tot_slots = int((klo_pos + khi_pos).sum() * P)

    # pack j-slots of consecutive positions into full MAXJ-slot calls,
    # one stream per bin (the gather base differs per bin)
    calls = []  # (binf, mcall, [(pos, j0_in_bin, off_in_call, m)])
    for binf, kvec in ((0, klo_pos), (1, khi_pos)):
        cur, fill = [], 0
        for pos in range(bpc):
            k = int(kvec[pos])
            j0 = 0
            while j0 < k:
                m = min(MAXJ - fill, k - j0)
                cur.append((pos, j0, fill, m))
                fill += m
                j0 += m
                if fill == MAXJ:
                    calls.append((binf, MAXJ, cur))
                    cur, fill = [], 0
        if fill:
            calls.append((binf, fill, cur))
    calls.sort(key=lambda cl: (min(s[0] for s in cl[2]), cl[0]))
    segs_pos = [[] for _ in range(bpc)]
    for cid, (binf, mc, segs) in enumerate(calls):
        for pos, j0, off, m in segs:
            segs_pos[pos].append((cid, off, j0, m, binf))
    for pos in range(bpc):
        selfbin = 0 if pos < npos_lo else 1
        segs_pos[pos].sort(key=lambda s: (s[4] != selfbin, s[2]))

    # per-(block, bin) gather index grids [k, P], padded by bin pad row
    idx_lo = {b: np.full((int(klo_pos[pos]), P), PADLO, np.int64)
              for pos in range(bpc) for b in blk_at[pos]}
    idx_hi = {b: np.full((int(khi_pos[pos]), P), PADHI - HIBASE, np.int64)
              for pos in range(bpc) for b in blk_at[pos]}
    # j index per edge: cumcount within (block, part, bin)
    key = (eblk * P + epart) * 2 + ishi
    eorder = np.argsort(key, kind="stable")
    ks = key[eorder]
    grp_start = np.flatnonzero(np.concatenate([[True], ks[1:] != ks[:-1]]))
    sizes = np.diff(np.concatenate([grp_start, [len(ks)]]))
    jvals = np.arange(len(ks)) - np.repeat(grp_start, sizes)
    ej = np.empty(len(ks), np.int64)
    ej[eorder] = jvals

    # edges land at j (+1 when their bin carries the self slot at j=0)
    lo_m = ishi == 0
    bs, ps, js, rs = eblk[lo_m], epart[lo_m], ej[lo_m], srow[lo_m]
    shift = (bs < NLO).astype(np.int64)
    for b in range(NBLK):
        m = bs == b
        idx_lo[b][js[m] + shift[m], ps[m]] = rs[m]
    hi_m = ishi == 1
    bs, ps, js, rs = eblk[hi_m], epart[hi_m], ej[hi_m], srow[hi_m] - HIBASE
    shift = (bs >= NLO).astype(np.int64)
    for b in range(NBLK):
        m = bs == b
        idx_hi[b][js[m] + shift[m], ps[m]] = rs[m]
    for b in range(NBLK):
        base = b * P
        selfidx = np.arange(base, base + P, dtype=np.int64)
        if b < NLO:
            idx_lo[b][0] = selfidx
        else:
            idx_hi[b][0] = selfidx - HIBASE

    # per-core concatenated idx16 in packed-call order
    core_idx = []
    for c in range(n_cores):
        chunks = []
        for binf, mc, segs in calls:
            arr = np.empty((mc, P), np.int64)
            for pos, j0, off, m in segs:
                b = int(blk_at[pos, c])
                grid = idx_lo[b] if binf == 0 else idx_hi[b]
                arr[off: off + m] = grid[j0: j0 + m]
            chunks.append(_idx16(arr.reshape(-1), mc))
        core_idx.append(np.ascontiguousarray(np.concatenate(chunks, axis=1)))
    tot8 = core_idx[0].shape[1]

    # weights: Wcat [256, 272] = [W.T | B_l | B_r]
    W = np.asarray(W, dtype=np.float32)
    attn_l = np.asarray(attn_l, dtype=np.float32).reshape(H, C)
    attn_r = np.asarray(attn_r, dtype=np.float32).reshape(H, C)
    A_l = np.zeros((HC, H), dtype=np.float32)
    A_r = np.zeros((HC, H), dtype=np.float32)
    for h in range(H):
        A_l[h * C: (h + 1) * C, h] = attn_l[h]
        A_r[h * C: (h + 1) * C, h] = attn_r[h]
    WT = np.ascontiguousarray(W.T)
    # xp columns stored c-major ([c][h]) so the per-edge t-broadcast
    # multiply has a stride-1 innermost dim (DVE 2x/4x packed mode)
    cperm = np.array([h * C + c for c in range(C) for h in range(H)])
    wcat = np.concatenate([WT[:, cperm], WT @ A_l, WT @ A_r], axis=1)
    wcat = np.ascontiguousarray(wcat.reshape(2, P, PSROW)).astype(np.float32)

    # x tiles, pre-permuted so the phase-A lhsT loads are contiguous:
    # xt[g, h, p, u*128+n] = x_slot[g*1024 + u*128 + n, h*128 + p]
    T = SLOTS // P
    x_slot = np.zeros((SLOTS, IN), dtype=np.float32)
    x_slot[row_of] = np.asarray(x, dtype=np.float32)
    xt = np.ascontiguousarray(
        x_slot.reshape(T // G8, G8, P, 2, P).transpose(0, 3, 4, 1, 2)
        .reshape(T // G8, 2, P, G8 * P), dtype=np.float32)

    alpad = np.full((2, 16), -80.0, dtype=ml_dtypes.bfloat16)
    ident = np.eye(P, dtype=ml_dtypes.bfloat16)

    meta = dict(n_cores=n_cores, T=T, bpc=bpc, row_of=row_of, blk_at=blk_at,
                calls=[(b, m) for b, m, _ in calls], segs_pos=segs_pos,
                tot8=tot8, tot_slots=tot_slots, cperm=cperm)
    shared = dict(xt=xt, wcat=wcat, alpad=alpad, ident=ident)
    per_core = [dict(idx=core_idx[c]) for c in range(n_cores)]
    return meta, shared, per_core


# ---------------------------------------------------------------- device IR
def _build_program(meta):
    import concourse.bacc as bacc
    import concourse.tile as tile
    from concourse import mybir

    _install_tile_patch()

    T, bpc, tot8 = meta["T"], meta["bpc"], meta["tot8"]
    calls, segs_pos = meta["calls"], meta["segs_pos"]
    n_cores = meta["n_cores"]
    npos_lo = NLO // n_cores
    f32 = mybir.dt.float32
    bf16 = mybir.dt.bfloat16
    i16 = mybir.dt.int16
    f32r = mybir.dt.float32r
    Alu = mybir.AluOpType
    Act = mybir.ActivationFunctionType

    nc = bacc.Bacc("TRN2", target_bir_lowering=False, debug=False,
                   num_devices=n_cores, num_swdge_queues=NQ)
    xt_in = nc.dram_tensor("xt", [T // G8, 2, P, G8 * P], f32r,
                           kind="ExternalInput").ap()
    wcat_in = nc.dram_tensor("wcat", [2, P, PSROW], f32r,
                             kind="ExternalInput").ap()
    alpad_in = nc.dram_tensor("alpad", [2, 16], bf16, kind="ExternalInput").ap()
    ident_in = nc.dram_tensor("ident", [P, P], bf16, kind="ExternalInput").ap()
    idx_in = nc.dram_tensor("idx", [P, tot8], i16, kind="ExternalInput").ap()
    out_ex = nc.dram_tensor("out", [bpc * P, HC], bf16,
                            kind="ExternalOutput").ap()

    with tile.TileContext(nc) as tc:
        with (
            tc.tile_pool(name="const", bufs=1) as cpool,
            tc.tile_pool(name="dram", bufs=1, space="DRAM") as dpool,
        ):
            table = dpool.tile([SLOTS, ROW], bf16)
            wc0 = cpool.tile([P, PSROW], f32r, tag="wc0")
            wc1 = cpool.tile([P, PSROW], f32r, tag="wc1")
            nc.sync.dma_start(wc0[:], wcat_in[0])
            nc.sync.dma_start(wc1[:], wcat_in[1])
            idt = cpool.tile([P, P], bf16, tag="idt")
            nc.sync.dma_start(idt[:], ident_in[:])
            alp = cpool.tile([2, 16], bf16, tag="alp")
            nc.sync.dma_start(alp[:], alpad_in[:])
            idx_t = cpool.tile([P, tot8], i16, tag="idx_t")
            nc.sync.dma_start(idx_t[:], idx_in[:])

            # ---- phase A: projection table
            with (
                tc.tile_pool(name="pa", bufs=3) as pa,
                tc.tile_pool(name="pa_ps", bufs=4, space="PSUM") as paps,
            ):
                for g in range(T // G8):
                    ld0 = pa.tile([P, G8, P], f32r, tag="ld0")
                    ld1 = pa.tile([P, G8, P], f32r, tag="ld1")
                    nc.sync.dma_start(
                        ld0[:].rearrange("p u n -> p (u n)"), xt_in[g, 0])
                    nc.sync.dma_start(
                        ld1[:].rearrange("p u n -> p (u n)"), xt_in[g, 1])
                    sbX = pa.tile([P, G8, RCOL], bf16, tag="sbX")
                    for u in range(G8):
                        ps = paps.tile([P, PSROW], f32)
                        nc.tensor.matmul(ps[:], lhsT=ld0[:, u, :],
                                         rhs=wc0[:], start=True, stop=False)
                        nc.tensor.matmul(ps[:], lhsT=ld1[:, u, :],
                                         rhs=wc1[:], start=False, stop=True)
                        nc.scalar.activation(out=sbX[:, u, :], in_=ps[:],
                                             func=Act.Copy)
                    dst = table[g * G8 * P: (g + 1) * G8 * P, :].rearrange(
                        "(u p) r -> p u r", p=P)
                    nc.sync.dma_start(dst[:, :, 0:RCOL], sbX[:])
            # patch pad-row attention logits to -80
            nc.sync.dma_start(table[PADLO: PADLO + 1, HC: HC + 16],
                              alp[0:1, :])
            nc.sync.dma_start(table[PADHI: PADHI + 1, HC: HC + 16],
                              alp[1:2, :])

            # ---- phase B: per dst-block gather + attention + accumulate
            with (
                tc.tile_pool(name="gat", bufs=10) as gp,
                tc.tile_pool(name="mt", bufs=6) as mp,
                tc.tile_pool(name="small", bufs=6) as sp,
                tc.tile_pool(name="ps", bufs=2, space="PSUM") as psp,
            ):
                qrr = [0]
                off8 = [0]
                call_tiles = {}
                next_call = [0]

                def gather_next():
                    cid = next_call[0]
                    binf, mc = calls[cid]
                    gt = gp.tile([P, MAXJ, ROW], bf16, tag="G")
                    src_ap = table[HIBASE:, :] if binf else table[:, :]
                    nc.gpsimd.dma_gather(
                        gt[:, 0:mc, :], src_ap,
                        idx_t[:, off8[0]: off8[0] + mc * 8],
                        mc * P, mc * P, ROW, queue_num=qrr[0])
                    qrr[0] = (qrr[0] + 1) % NQ
                    off8[0] += mc * 8
                    call_tiles[cid] = gt
                    next_call[0] += 1

                for pos in range(bpc):
                    segs = segs_pos[pos]
                    needed = max(s[0] for s in segs)
                    while next_call[0] <= needed:
                        gather_next()
                    U2 = psp.tile([P, 2 * HC], f32)
                    den = sp.tile([P, H], f32, tag="den")
                    cid0, off0 = segs[0][0], segs[0][1]
                    ar_bc = call_tiles[cid0][:, off0: off0 + 1,
                                             HC + H: HC + 2 * H]
                    mmops = []  # (mt_tile, jj, width)
                    first_seg = True
                    for cid, off, j0, m, binf in segs:
                        gt = call_tiles[cid]
                        sl = slice(off, off + m)
                        lg = sp.tile([P, MAXJ, H], f32, tag="lg")
                        nc.vector.tensor_tensor(
                            out=lg[:, 0:m, :],
                            in0=gt[:, sl, HC: HC + H],
                            in1=ar_bc.to_broadcast([P, m, H]),
                            op=Alu.add)
                        mt = mp.tile([P, MAXJ, HC + H], bf16, tag="MT")
                        e2 = sp.tile([P, MAXJ, H], bf16, tag="e2")
                        nc.scalar.activation(out=mt[:, 0:m, HC: HC + H],
                                             in_=lg[:, 0:m, :], func=Act.Exp)
                        nc.scalar.activation(out=e2[:, 0:m, :],
                                             in_=lg[:, 0:m, :], func=Act.Exp,
                                             scale=0.2)
                        nc.vector.tensor_tensor(
                            out=mt[:, 0:m, HC: HC + H],
                            in0=mt[:, 0:m, HC: HC + H],
                            in1=e2[:, 0:m, :], op=Alu.max)
                        nc.vector.tensor_tensor(
                            out=mt[:, 0:m, 0:HC].rearrange(
                                "p m (c h) -> p m c h", h=H),
                            in0=gt[:, sl, 0:HC].rearrange(
                                "p m (c h) -> p m c h", h=H),
                            in1=mt[:, 0:m, HC: HC + H].unsqueeze(2)
                                .to_broadcast([P, m, C, H]),
                            op=Alu.mult)
                        # denominator on DVE: reduce t over j, accumulate
                        dseg = sp.tile([P, H], f32, tag="dseg")
                        nc.vector.tensor_reduce(
                            dseg[:],
                            mt[:, 0:m, HC: HC + H].rearrange("p m h -> p h m"),
                            mybir.AxisListType.X, Alu.add)
                        if first_seg:
                            nc.vector.tensor_scalar(den[:], dseg[:], 1e-6,
                                                    None, Alu.max)
                            first_seg = False
                        else:
                            nc.vector.tensor_tensor(out=den[:], in0=den[:],
                                                    in1=dseg[:], op=Alu.add)
                        for jj in range(0, m, 2):
                            w = 2 if jj + 1 < m else 1
                            mmops.append((mt, jj, w))
                    k2 = next((i for i, o in enumerate(mmops) if o[2] == 2), 0)
                    if k2:
                        mmops[0], mmops[k2] = mmops[k2], mmops[0]
                    haspair = mmops[0][2] == 2
                    for i, (mt, jj, w) in enumerate(mmops):
                        nc.tensor.matmul(
                            U2[:, 0: w * HC],
                            lhsT=idt[:],
                            rhs=mt[:, jj: jj + w, 0:HC],
                            start=(i == 0), stop=(i == len(mmops) - 1),
                            skip_group_check=True)
                    rec = sp.tile([P, H], bf16, tag="rec")
                    with nc.allow_low_precision(
                            reason="bf16 reciprocal feeds bf16 output"):
                        nc.vector.reciprocal(rec[:], den[:])
                    us = sp.tile([P, HC], bf16, tag="us")
                    with nc.allow_low_precision(
                            reason="final per-block fold; output is bf16"):
                        if haspair:
                            u1 = sp.tile([P, HC], f32, tag="u1")
                            nc.scalar.activation(out=u1[:],
                                                 in_=U2[:, HC: 2 * HC],
                                                 func=Act.Copy)
                            nc.vector.tensor_tensor(out=us[:], in0=u1[:],
                                                    in1=U2[:, 0:HC],
                                                    op=Alu.add)
                        else:
                            nc.vector.tensor_scalar(us[:], U2[:, 0:HC], 1.0,
                                                    None, Alu.mult)
                    ob = sp.tile([P, HC], bf16, tag="ob")
                    nc.vector.tensor_tensor(
                        out=ob[:].rearrange("p (c h) -> p c h", h=H),
                        in0=us[:].rearrange("p (c h) -> p c h", h=H),
                        in1=rec[:].unsqueeze(1).to_broadcast([P, C, H]),
                        op=Alu.mult)
                    nc.sync.dma_start(out_ex[pos * P: (pos + 1) * P, :], ob[:])
    nc.compile()
    return nc


# ---------------------------------------------------------------- runner
def _run(inputs, trace=False, n_cores=8):
    from concourse.bass_utils import run_bass_kernel_spmd

    x = np.asarray(inputs["x"])
    edge_index = np.asarray(inputs["edge_index"])
    meta, shared, per_core = _preprocess(
        x, edge_index, inputs["W"], inputs["attn_l"], inputs["attn_r"], n_cores
    )
    nc = _build_program(meta)
    in_maps = [{**shared, **pc} for pc in per_core]
    res = run_bass_kernel_spmd(nc, in_maps, list(range(n_cores)), trace=trace)

    # reassemble: block at (pos, core) covers table rows [b*128, b*128+128)
    blk_at = meta["blk_at"]
    bpc, row_of = meta["bpc"], meta["row_of"]
    full = np.zeros((SLOTS, HC), np.float32)
    for c in range(n_cores):
        shard = np.asarray(res.results[c]["out"], dtype=np.float32)
        for pos in range(bpc):
            b = int(blk_at[pos, c])
            full[b * P: (b + 1) * P] = shard[pos * P: (pos + 1) * P]
    out = np.empty((row_of.shape[0], HC), np.float32)
    out[:, meta["cperm"]] = full[row_of]  # undo c-major column layout
    return np.ascontiguousarray(out), res, meta


def kernel(**inputs) -> np.ndarray:
    out, _, _ = _run(inputs, trace=False)
    return out


# revision 32
# speedup vs baseline: 2.3045x; 1.0476x over previous
"""CustomGAT (gnn_message_passing) Trainium2 kernel — 8-core SPMD, v2.

Design (dst-resident, free-axis edge layout):
  * Host: order nodes by (lo-deg, hi-deg) lexsort so consecutive 128-node
    dst blocks have near-equal per-bin in-degrees (few % slot padding).
    Edges of dst p live on partition p along the free axis (j-slots);
    per-edge src table rows split into two int16-addressable bins
    (rows < 32768, base 0; rows >= 32768, base 17408).
  * Device phase A (replicated on all cores): projection table rows
    [xp bf16 x256 | al bf16 x8 | ar bf16 x8 | pad to 768B] via fp32r
    matmuls; one ACT copy per tile PSUM->SBUF. Pad rows 0 and 50001 get
    al=ar=-80 so padded gather slots contribute exp(lrelu(-160+..)) ~ 0.
  * Device phase B (49 dst blocks per core): one m=1 "self" gather call
    brings each dst's own row (= the self-loop edge AND the block's ar
    column); real edges batch-gathered by the GPSIMD dma_gather ucode
    round-robined over 4 SWDGE queues (the descriptor-generation
    bottleneck measured 8.5 ns/row on one queue, ~3.3 ns/row on four).
    Logits al+ar on DVE; exp(leaky_relu(x)) = max(exp(x), exp(0.2x)) via
    two ACT exps + DVE max; messages xp*t on DVE; j-slot reduction via
    identity matmuls accumulating into PSUM [128, 264] whose last 8 cols
    collect the softmax denominator; one normalize per block.
  * Host: inverse-permute per-core shards to the full [N, 256] output.
"""

import numpy as np

# ---------------------------------------------------------------- constants
H = 8
C = 32
HC = H * C  # 256
IN = 256
P = 128
ROW = 384   # bf16 row: [xp 0:256 | al 256:264 | ar 264:272 | pad]
RCOL = 272  # used row columns
PSROW = 272
SLOTS = 50176  # 392 * 128; row 0 = lo pad, rows 50001+ = hi pad area
NBLK = SLOTS // P  # 392
NLO = 256   # blocks 0..255 have rows < 32768
LOWMAX = 32768
HIBASE = 17408  # hi bin rows [32768, 50176) addressed as row - 17408
PADLO = 0
PADHI = 50001
MAXJ = 8  # j-slots per dma_gather call (1024-index ucode cap)
NQ = 4    # SWDGE queues
G8 = 8    # phase A tiles per DMA group


# ---------------------------------------------------------------- tile patch
def _install_tile_patch():
    """The axon-path walrus rejects >2 sync waits on one instruction; split
    the TileContext tail-drain waits into one carrier drain per proc."""
    import concourse.tile as tile
    from concourse.vector_clock import ScopedClock, VectorClock

    if getattr(tile.TileContext, "_drain_patch_installed", False):
        return

    def _drain_and_barrier(self, tick_clock, wait_clock):
        gc = tick_clock.global_clock
        n = len(gc)
        for p in range(n):
            if gc[p] == 0:
                continue
            req = VectorClock([gc[q] if q == p else 0 for q in range(n)])
            d = self.nc.sync.drain()
            wait_clock.add_sem_waits(d.ins, ScopedClock({None: req}))
        self.nc.all_engine_barrier()
        assert self.sems is not None
        popped = self.nc._tile_sem_poison_stack.pop()
        assert popped is self._sem_poison
        self.nc.clear_and_free_semaphores(list(self.sems.allocated().values()))
        self.nc.all_engine_barrier()

    tile.TileContext._drain_and_barrier = _drain_and_barrier
    tile.TileContext._drain_patch_installed = True


# ---------------------------------------------------------------- host prep
def _idx16(vals, m):
    """Encode row indices for dma_gather: [128, m*8] int16, index k at
    [k%16 (+16*rep), k//16], replicated for the 8 Q7 cores."""
    enc = vals.astype(np.int64).astype(np.int16)
    a = enc.reshape(m * 8, 16).T  # [16, m*8]
    return np.tile(a, (8, 1))


def _calls_of(k):
    return [min(MAXJ, k - s) for s in range(0, k, MAXJ)]


def _preprocess(x, edge_index, W, attn_l, attn_r, n_cores):
    import ml_dtypes

    N = x.shape[0]
    src = np.asarray(edge_index[0]).astype(np.int64)
    dst = np.asarray(edge_index[1]).astype(np.int64)

    deg = np.bincount(dst, minlength=N)
    # membership of the lo row range is fixed up front (top in-degree nodes)
    # so each edge's bin never moves; then sort within each set by
    # (lo-deg, hi-deg) for near-uniform per-block per-bin degrees.
    S_nodes = np.argsort(-deg, kind="stable")[: LOWMAX - 1]
    inS = np.zeros(N, bool)
    inS[S_nodes] = True
    lo = np.bincount(dst[inS[src]], minlength=N)
    hi = deg - lo
    T_nodes = np.flatnonzero(~inS)
    Ssort = S_nodes[np.lexsort((hi[S_nodes], lo[S_nodes]))]
    Tsort = T_nodes[np.lexsort((hi[T_nodes], lo[T_nodes]))]
    row_of = np.empty(N, np.int64)
    row_of[Ssort] = 1 + np.arange(len(Ssort))  # row 0 reserved for lo-pad
    row_of[Tsort] = LOWMAX + np.arange(len(Tsort))

    srow = row_of[src]
    drow = row_of[dst]
    ishi = (srow >= LOWMAX).astype(np.int64)

    eblk = drow // P
    epart = drow % P
    cnt = np.zeros((NBLK, P, 2), np.int64)
    np.add.at(cnt, (eblk, epart, ishi), 1)
    klo_b = cnt[:, :, 0].max(axis=1)
    khi_b = cnt[:, :, 1].max(axis=1)
    w_b = klo_b + khi_b

    # positions: 0..31 lo-group (blocks 0..255), 32..48 hi-group.
    # 8 CONSECUTIVE lexsorted blocks share a position so their (klo, khi)
    # nearly coincide and the per-position max costs ~1 extra slot.
    bpc = NBLK // n_cores  # 49
    npos_lo = NLO // n_cores  # 32
    blk_at = np.arange(NBLK, dtype=np.int64).reshape(bpc, n_cores)

    # uniform per-position call shapes (max over the 8 cores' blocks);
    # the self row is slot j=0 of the block's own bin (lo for pos<npos_lo)
    klo_pos = klo_b[blk_at].max(axis=1)
    khi_pos = khi_b[blk_at].max(axis=1)
    klo_pos[:npos_lo] += 1
    khi_pos[npos_lo:] += 1
    tot_slots = int((klo_pos + khi_pos).sum() * P)

    # pack j-slots of consecutive positions into full MAXJ-slot calls,
    # one stream per bin (the gather base differs per bin)
    calls = []  # (binf, mcall, [(pos, j0_in_bin, off_in_call, m)])
    for binf, kvec in ((0, klo_pos), (1, khi_pos)):
        cur, fill = [], 0
        for pos in range(bpc):
            k = int(kvec[pos])
            j0 = 0
            while j0 < k:
                m = min(MAXJ - fill, k - j0)
                cur.append((pos, j0, fill, m))
                fill += m
                j0 += m
                if fill == MAXJ:
                    calls.append((binf, MAXJ, cur))
                    cur, fill = [], 0
        if fill:
            calls.append((binf, fill, cur))
    calls.sort(key=lambda cl: (min(s[0] for s in cl[2]), cl[0]))
    segs_pos = [[] for _ in range(bpc)]
    for cid, (binf, mc, segs) in enumerate(calls):
        for pos, j0, off, m in segs:
            segs_pos[pos].append((cid, off, j0, m, binf))
    for pos in range(bpc):
        selfbin = 0 if pos < npos_lo else 1
        segs_pos[pos].sort(key=lambda s: (s[4] != selfbin, s[2]))

    # per-(block, bin) gather index grids [k, P], padded by bin pad row
    idx_lo = {b: np.full((int(klo_pos[pos]), P), PADLO, np.int64)
              for pos in range(bpc) for b in blk_at[pos]}
    idx_hi = {b: np.full((int(khi_pos[pos]), P), PADHI - HIBASE, np.int64)
              for pos in range(bpc) for b in blk_at[pos]}
    # j index per edge: cumcount within (block, part, bin)
    key = (eblk * P + epart) * 2 + ishi
    eorder = np.argsort(key, kind="stable")
    ks = key[eorder]
    grp_start = np.flatnonzero(np.concatenate([[True], ks[1:] != ks[:-1]]))
    sizes = np.diff(np.concatenate([grp_start, [len(ks)]]))
    jvals = np.arange(len(ks)) - np.repeat(grp_start, sizes)
    ej = np.empty(len(ks), np.int64)
    ej[eorder] = jvals

    # edges land at j (+1 when their bin carries the self slot at j=0)
    lo_m = ishi == 0
    bs, ps, js, rs = eblk[lo_m], epart[lo_m], ej[lo_m], srow[lo_m]
    shift = (bs < NLO).astype(np.int64)
    for b in range(NBLK):
        m = bs == b
        idx_lo[b][js[m] + shift[m], ps[m]] = rs[m]
    hi_m = ishi == 1
    bs, ps, js, rs = eblk[hi_m], epart[hi_m], ej[hi_m], srow[hi_m] - HIBASE
    shift = (bs >= NLO).astype(np.int64)
    for b in range(NBLK):
        m = bs == b
        idx_hi[b][js[m] + shift[m], ps[m]] = rs[m]
    for b in range(NBLK):
        base = b * P
        selfidx = np.arange(base, base + P, dtype=np.int64)
        if b < NLO:
            idx_lo[b][0] = selfidx
        else:
            idx_hi[b][0] = selfidx - HIBASE

    # per-core concatenated idx16 in packed-call order
    core_idx = []
    for c in range(n_cores):
        chunks = []
        for binf, mc, segs in calls:
            arr = np.empty((mc, P), np.int64)
            for pos, j0, off, m in segs:
                b = int(blk_at[pos, c])
                grid = idx_lo[b] if binf == 0 else idx_hi[b]
                arr[off: off + m] = grid[j0: j0 + m]
            chunks.append(_idx16(arr.reshape(-1), mc))
        core_idx.append(np.ascontiguousarray(np.concatenate(chunks, axis=1)))
    tot8 = core_idx[0].shape[1]

    # weights: Wcat [256, 272] = [W.T | B_l | B_r]
    W = np.asarray(W, dtype=np.float32)
    attn_l = np.asarray(attn_l, dtype=np.float32).reshape(H, C)
    attn_r = np.asarray(attn_r, dtype=np.float32).reshape(H, C)
    A_l = np.zeros((HC, H), dtype=np.float32)
    A_r = np.zeros((HC, H), dtype=np.float32)
    for h in range(H):
        A_l[h * C: (h + 1) * C, h] = attn_l[h]
        A_r[h * C: (h + 1) * C, h] = attn_r[h]
    WT = np.ascontiguousarray(W.T)
    # xp columns stored c-major ([c][h]) so the per-edge t-broadcast
    # multiply has a stride-1 innermost dim (DVE 2x/4x packed mode)
    cperm = np.array([h * C + c for c in range(C) for h in range(H)])
    wcat = np.concatenate([WT[:, cperm], WT @ A_l, WT @ A_r], axis=1)
    wcat = np.ascontiguousarray(wcat.reshape(2, P, PSROW)).astype(ml_dtypes.bfloat16)

    # x tiles, pre-permuted so the phase-A lhsT loads are contiguous:
    # xt[g, h, p, u*128+n] = x_slot[g*1024 + u*128 + n, h*128 + p]
    T = SLOTS // P
    x_slot = np.zeros((SLOTS, IN), dtype=np.float32)
    x_slot[row_of] = np.asarray(x, dtype=np.float32)
    xt = np.ascontiguousarray(
        x_slot.reshape(T // G8, G8, P, 2, P).transpose(0, 3, 4, 1, 2)
        .reshape(T // G8, 2, P, G8 * P)).astype(ml_dtypes.bfloat16)

    alpad = np.full((2, 16), -80.0, dtype=ml_dtypes.bfloat16)
    ident = np.eye(P, dtype=ml_dtypes.bfloat16)

    meta = dict(n_cores=n_cores, T=T, bpc=bpc, row_of=row_of, blk_at=blk_at,
                calls=[(b, m) for b, m, _ in calls], segs_pos=segs_pos,
                tot8=tot8, tot_slots=tot_slots, cperm=cperm)
    shared = dict(xt=xt, wcat=wcat, alpad=alpad, ident=ident)
    per_core = [dict(idx=core_idx[c]) for c in range(n_cores)]
    return meta, shared, per_core


# ---------------------------------------------------------------- device IR
def _build_program(meta):
    import concourse.bacc as bacc
    import concourse.tile as tile
    from concourse import mybir

    _install_tile_patch()

    T, bpc, tot8 = meta["T"], meta["bpc"], meta["tot8"]
    calls, segs_pos = meta["calls"], meta["segs_pos"]
    n_cores = meta["n_cores"]
    npos_lo = NLO // n_cores
    f32 = mybir.dt.float32
    bf16 = mybir.dt.bfloat16
    i16 = mybir.dt.int16
    f32r = mybir.dt.float32r
    Alu = mybir.AluOpType
    Act = mybir.ActivationFunctionType

    nc = bacc.Bacc("TRN2", target_bir_lowering=False, debug=False,
                   num_devices=n_cores, num_swdge_queues=NQ)
    xt_in = nc.dram_tensor("xt", [T // G8, 2, P, G8 * P], bf16,
                           kind="ExternalInput").ap()
    wcat_in = nc.dram_tensor("wcat", [2, P, PSROW], bf16,
                             kind="ExternalInput").ap()
    alpad_in = nc.dram_tensor("alpad", [2, 16], bf16, kind="ExternalInput").ap()
    ident_in = nc.dram_tensor("ident", [P, P], bf16, kind="ExternalInput").ap()
    idx_in = nc.dram_tensor("idx", [P, tot8], i16, kind="ExternalInput").ap()
    out_ex = nc.dram_tensor("out", [bpc * P, HC], bf16,
                            kind="ExternalOutput").ap()

    with tile.TileContext(nc) as tc:
        with (
            tc.tile_pool(name="const", bufs=1) as cpool,
            tc.tile_pool(name="dram", bufs=1, space="DRAM") as dpool,
        ):
            table = dpool.tile([SLOTS, ROW], bf16)
            wc0 = cpool.tile([P, PSROW], bf16, tag="wc0")
            wc1 = cpool.tile([P, PSROW], bf16, tag="wc1")
            nc.sync.dma_start(wc0[:], wcat_in[0])
            nc.sync.dma_start(wc1[:], wcat_in[1])
            idt = cpool.tile([P, P], bf16, tag="idt")
            nc.sync.dma_start(idt[:], ident_in[:])
            alp = cpool.tile([2, 16], bf16, tag="alp")
            nc.sync.dma_start(alp[:], alpad_in[:])
            idx_t = cpool.tile([P, tot8], i16, tag="idx_t")
            nc.sync.dma_start(idx_t[:], idx_in[:])

            # ---- phase A: projection table
            with (
                tc.tile_pool(name="pa", bufs=3) as pa,
                tc.tile_pool(name="pa_ps", bufs=4, space="PSUM") as paps,
            ):
                for g in range(T // G8):
                    ld0 = pa.tile([P, G8, P], bf16, tag="ld0")
                    ld1 = pa.tile([P, G8, P], bf16, tag="ld1")
                    nc.sync.dma_start(
                        ld0[:].rearrange("p u n -> p (u n)"), xt_in[g, 0])
                    nc.sync.dma_start(
                        ld1[:].rearrange("p u n -> p (u n)"), xt_in[g, 1])
                    sbX = pa.tile([P, G8, RCOL], bf16, tag="sbX")
                    for u in range(G8):
                        ps = paps.tile([P, PSROW], f32)
                        nc.tensor.matmul(ps[:], lhsT=ld0[:, u, :],
                                         rhs=wc0[:], start=True, stop=False)
                        nc.tensor.matmul(ps[:], lhsT=ld1[:, u, :],
                                         rhs=wc1[:], start=False, stop=True)
                        nc.scalar.activation(out=sbX[:, u, :], in_=ps[:],
                                             func=Act.Copy)
                    dst = table[g * G8 * P: (g + 1) * G8 * P, :].rearrange(
                        "(u p) r -> p u r", p=P)
                    nc.sync.dma_start(dst[:, :, 0:RCOL], sbX[:])
            # patch pad-row attention logits to -80
            nc.sync.dma_start(table[PADLO: PADLO + 1, HC: HC + 16],
                              alp[0:1, :])
            nc.sync.dma_start(table[PADHI: PADHI + 1, HC: HC + 16],
                              alp[1:2, :])

            # ---- phase B: per dst-block gather + attention + accumulate
            with (
                tc.tile_pool(name="gat", bufs=10) as gp,
                tc.tile_pool(name="mt", bufs=6) as mp,
                tc.tile_pool(name="small", bufs=6) as sp,
                tc.tile_pool(name="ps", bufs=2, space="PSUM") as psp,
            ):
                qrr = [0]
                off8 = [0]
                call_tiles = {}
                next_call = [0]

                def gather_next():
                    cid = next_call[0]
                    binf, mc = calls[cid]
                    gt = gp.tile([P, MAXJ, ROW], bf16, tag="G")
                    src_ap = table[HIBASE:, :] if binf else table[:, :]
                    nc.gpsimd.dma_gather(
                        gt[:, 0:mc, :], src_ap,
                        idx_t[:, off8[0]: off8[0] + mc * 8],
                        mc * P, mc * P, ROW, queue_num=qrr[0])
                    qrr[0] = (qrr[0] + 1) % NQ
                    off8[0] += mc * 8
                    call_tiles[cid] = gt
                    next_call[0] += 1

                for pos in range(bpc):
                    segs = segs_pos[pos]
                    needed = max(s[0] for s in segs)
                    while next_call[0] <= needed:
                        gather_next()
                    U2 = psp.tile([P, 2 * HC], f32)
                    den = sp.tile([P, H], f32, tag="den")
                    cid0, off0 = segs[0][0], segs[0][1]
                    ar_bc = call_tiles[cid0][:, off0: off0 + 1,
                                             HC + H: HC + 2 * H]
                    mmops = []  # (mt_tile, jj, width)
                    first_seg = True
                    for cid, off, j0, m, binf in segs:
                        gt = call_tiles[cid]
                        sl = slice(off, off + m)
                        lg = sp.tile([P, MAXJ, H], f32, tag="lg")
                        nc.vector.tensor_tensor(
                            out=lg[:, 0:m, :],
                            in0=gt[:, sl, HC: HC + H],
                            in1=ar_bc.to_broadcast([P, m, H]),
                            op=Alu.add)
                        mt = mp.tile([P, MAXJ, HC + H], bf16, tag="MT")
                        e2 = sp.tile([P, MAXJ, H], bf16, tag="e2")
                        nc.scalar.activation(out=mt[:, 0:m, HC: HC + H],
                                             in_=lg[:, 0:m, :], func=Act.Exp)
                        nc.scalar.activation(out=e2[:, 0:m, :],
                                             in_=lg[:, 0:m, :], func=Act.Exp,
                                             scale=0.2)
                        nc.vector.tensor_tensor(
                            out=mt[:, 0:m, HC: HC + H],
                            in0=mt[:, 0:m, HC: HC + H],
                            in1=e2[:, 0:m, :], op=Alu.max)
                        nc.vector.tensor_tensor(
                            out=mt[:, 0:m, 0:HC].rearrange(
                                "p m (c h) -> p m c h", h=H),
                            in0=gt[:, sl, 0:HC].rearrange(
                                "p m (c h) -> p m c h", h=H),
                            in1=mt[:, 0:m, HC: HC + H].unsqueeze(2)
                                .to_broadcast([P, m, C, H]),
                            op=Alu.mult)
                        # denominator on DVE: reduce t over j, accumulate
                        dseg = sp.tile([P, H], f32, tag="dseg")
                        nc.vector.tensor_reduce(
                            dseg[:],
                            mt[:, 0:m, HC: HC + H].rearrange("p m h -> p h m"),
                            mybir.AxisListType.X, Alu.add)
                        if first_seg:
                            nc.vector.tensor_scalar(den[:], dseg[:], 1e-6,
                                                    None, Alu.max)
                            first_seg = False
                        else:
                            nc.vector.tensor_tensor(out=den[:], in0=den[:],
                                                    in1=dseg[:], op=Alu.add)
                        for jj in range(0, m, 2):
                            w = 2 if jj + 1 < m else 1
                            mmops.append((mt, jj, w))
                    k2 = next((i for i, o in enumerate(mmops) if o[2] == 2), 0)
                    if k2:
                        mmops[0], mmops[k2] = mmops[k2], mmops[0]
                    haspair = mmops[0][2] == 2
                    for i, (mt, jj, w) in enumerate(mmops):
                        nc.tensor.matmul(
                            U2[:, 0: w * HC],
                            lhsT=idt[:],
                            rhs=mt[:, jj: jj + w, 0:HC],
                            start=(i == 0), stop=(i == len(mmops) - 1),
                            skip_group_check=True)
                    rec = sp.tile([P, H], bf16, tag="rec")
                    with nc.allow_low_precision(
                            reason="bf16 reciprocal feeds bf16 output"):
                        nc.vector.reciprocal(rec[:], den[:])
                    us = sp.tile([P, HC], bf16, tag="us")
                    with nc.allow_low_precision(
                            reason="final per-block fold; output is bf16"):
                        if haspair:
                            u1 = sp.tile([P, HC], f32, tag="u1")
                            nc.scalar.activation(out=u1[:],
                                                 in_=U2[:, HC: 2 * HC],
                                                 func=Act.Copy)
                            nc.vector.tensor_tensor(out=us[:], in0=u1[:],
                                                    in1=U2[:, 0:HC],
                                                    op=Alu.add)
                        else:
                            nc.vector.tensor_scalar(us[:], U2[:, 0:HC], 1.0,
                                                    None, Alu.mult)
                    ob = sp.tile([P, HC], bf16, tag="ob")
                    nc.vector.tensor_tensor(
                        out=ob[:].rearrange("p (c h) -> p c h", h=H),
                        in0=us[:].rearrange("p (c h) -> p c h", h=H),
                        in1=rec[:].unsqueeze(1).to_broadcast([P, C, H]),
                        op=Alu.mult)
                    nc.sync.dma_start(out_ex[pos * P: (pos + 1) * P, :], ob[:])
    nc.compile()
    return nc


# ---------------------------------------------------------------- runner
def _run(inputs, trace=False, n_cores=8):
    from concourse.bass_utils import run_bass_kernel_spmd

    x = np.asarray(inputs["x"])
    edge_index = np.asarray(inputs["edge_index"])
    meta, shared, per_core = _preprocess(
        x, edge_index, inputs["W"], inputs["attn_l"], inputs["attn_r"], n_cores
    )
    nc = _build_program(meta)
    in_maps = [{**shared, **pc} for pc in per_core]
    res = run_bass_kernel_spmd(nc, in_maps, list(range(n_cores)), trace=trace)

    # reassemble: block at (pos, core) covers table rows [b*128, b*128+128)
    blk_at = meta["blk_at"]
    bpc, row_of = meta["bpc"], meta["row_of"]
    full = np.zeros((SLOTS, HC), np.float32)
    for c in range(n_cores):
        shard = np.asarray(res.results[c]["out"], dtype=np.float32)
        for pos in range(bpc):
            b = int(blk_at[pos, c])
            full[b * P: (b + 1) * P] = shard[pos * P: (pos + 1) * P]
    out = np.empty((row_of.shape[0], HC), np.float32)
    out[:, meta["cperm"]] = full[row_of]  # undo c-major column layout
    return np.ascontiguousarray(out), res, meta


def kernel(**inputs) -> np.ndarray:
    out, _, _ = _run(inputs, trace=False)
    return out
